# revision 6
# baseline (speedup 1.0000x reference)
"""Causal multi-head attention on 8 TRN2 NeuronCores.

Sharding: core c = (batch b=c//2, head-group g=c%2). Each core computes QKV
projections + causal attention for its 8 heads over the full sequence of its
batch; 2-rank AllGathers (pair shares a batch) exchange attention outputs;
each core then runs the output projection for its half of the output columns.

Perf structure (v2):
  - DMAs consolidated into multi-dim-AP transfers and split across the two
    HW DGE queues (sync + scalar) so the PE never waits on a serialized
    descriptor chain.
  - v_aug ones/zeros pattern built by gpsimd memsets (no DMA).
  - Causal mask multiply shrunk to the 128-wide diagonal triangle (the mask
    is identical for every diagonal block).
  - Softmax normalize runs early (emitted at block 2 of the following chunk)
    straight from PSUM -- no staging copy; reciprocal on DVE, partition
    broadcast on gpsimd.
  - Last pair processes q-chunks in descending order with one AllGather per
    chunk; output projection phase A covers blocks [0,1,4,5,2,6] so phase B
    is just [3,7]+store, emitted as fillers as each AllGather lands.
  - Output stored as bf16, stores alternate between the two DMA queues.
"""

import numpy as np
import ml_dtypes

import concourse.bass as bass
import concourse.mybir as mybir
import concourse.tile as tile
from concourse import bacc
from concourse import bass_utils

BF16 = mybir.dt.bfloat16
F32 = mybir.dt.float32

B, S, D = 4, 2048, 1024
H, DK = 16, 64
HPG = 8          # heads per group (per core)
DG = HPG * DK    # 512, d-range per core
NPAIR = 4        # head pairs per core
SC = 512         # sequence chunk (matmul free dim)
NSC = S // SC    # 4
KB = 128         # key block
NKB = S // KB    # 16
P = 128
NI = D // P      # 8

_cache = {}
DEBUG = False


def _build():
    nc = bacc.Bacc("TRN2", target_bir_lowering=False, debug=False, num_devices=8)

    xT = nc.dram_tensor("xT", [D, S], BF16, kind="ExternalInput")
    wqT = nc.dram_tensor("wqT", [D, DG], BF16, kind="ExternalInput")
    wkT = nc.dram_tensor("wkT", [D, DG], BF16, kind="ExternalInput")
    wvT = nc.dram_tensor("wvT", [D, DG], BF16, kind="ExternalInput")
    woT = nc.dram_tensor("woT", [D, DG], BF16, kind="ExternalInput")
    # bconst: bq [0:4], bk [4:8], bv broadcast [8:520], bo broadcast [520:1032]
    bconst = nc.dram_tensor("bconst", [P, 8 + 2 * DG], F32, kind="ExternalInput")
    maskd = nc.dram_tensor("maskd", [P, KB], BF16, kind="ExternalInput")
    out = nc.dram_tensor("out", [S, DG], BF16, kind="ExternalOutput")

    with tile.TileContext(nc) as tc:
        _emit(nc, tc, xT, wqT, wkT, wvT, woT, bconst, maskd, out)
    nc.compile()
    return nc


def _emit(nc, tc, xT, wqT, wkT, wvT, woT, bconst_d, maskd, out):
    ctxs = []

    def pool(name, bufs, space="SBUF"):
        cm = tc.tile_pool(name=name, bufs=bufs, space=space)
        p = cm.__enter__()
        ctxs.append(cm)
        return p

    const = pool("const", 1)
    dram = pool("dram", 1, space="DRAM")
    qk_pool = pool("qk", 2)
    att_pool = pool("att", 2)
    exp_pool = pool("exp", 5)
    small = pool("small", 3)
    out_pool = pool("outp", 2)
    ps_qk = pool("ps_qk", 2, space="PSUM")
    ps_sc = pool("ps_sc", 2, space="PSUM")
    ps_av = pool("ps_av", 2, space="PSUM")

    # ---- constants / weights ----
    xt = const.tile([P, NI, S], BF16, name="xt")
    wq = const.tile([P, NI, DG], BF16, name="wq")
    wk = const.tile([P, NI, DG], BF16, name="wk")
    wv = const.tile([P, NI, DG], BF16, name="wv")
    wo = const.tile([P, NI, DG], BF16, name="wo")
    bconst = const.tile([P, 8 + 2 * DG], F32, name="bconst")
    mask128 = const.tile([P, 1, KB], BF16, name="mask128")
    # v_aug[:, t, h, :]: col 0 = ones (softmax denominator row), cols 1:64 =
    # zeros (PSUM partition alignment pad), cols 64:128 = v
    v_aug = const.tile([P, NKB, HPG, P], BF16, name="v_aug")

    # weights path on the scalar HW DGE queue; x chunks on the sync queue
    nc.scalar.dma_start(wq[:], wqT.rearrange("(i p) d -> p i d", p=P))
    nc.scalar.dma_start(wk[:], wkT.rearrange("(i p) d -> p i d", p=P))
    xTr = xT.rearrange("(i p) s -> p i s", p=P)
    for sc in range(NSC):
        nc.sync.dma_start(xt[:, :, SC * sc:SC * (sc + 1)],
                          xTr[:, :, SC * sc:SC * (sc + 1)])
    nc.scalar.dma_start(wv[:], wvT.rearrange("(i p) d -> p i d", p=P))
    nc.scalar.dma_start(bconst[:], bconst_d[:])
    nc.scalar.dma_start(mask128[:], maskd[:])
    nc.scalar.dma_start(wo[:], woT.rearrange("(i p) d -> p i d", p=P))

    # v_aug constant pattern via gpsimd (keeps both DMA queues clear)
    for t in range(NKB):
        nc.gpsimd.memset(v_aug[:, t, :, 1:DK], 0.0)
        nc.gpsimd.memset(v_aug[:, t, :, 0:1], 1.0)

    bq_t = bconst[:, 0:4]
    bk_t = bconst[:, 4:8]
    bv_t = bconst[:, 8:8 + DG]
    bo_t = bconst[:, 8 + DG:8 + 2 * DG]

    # DRAM bounce buffers for the pairwise AllGathers
    agin = dram.tile([3, P, S], BF16, name="agin")        # pairs 0-2, full S
    agin3 = dram.tile([NSC, P, SC], BF16, name="agin3")   # pair 3, per chunk
    agout0 = dram.tile([2, 2, P, S], BF16, name="agout0")  # pairs 0-1
    agout1 = dram.tile([2, P, S], BF16, name="agout1")     # pair 2
    agout2 = dram.tile([NSC, 2, P, SC], BF16, name="agout2")  # pair 3 chunks

    groups = [[0, 1], [2, 3], [4, 5], [6, 7]]

    qT_pair = [qk_pool.tile([P, S], BF16, tag="qT", name=f"qTp{pp}")
               for pp in range(NPAIR)]
    kT_pair = [qk_pool.tile([P, S], BF16, tag="kT", name=f"kTp{pp}")
               for pp in range(NPAIR)]

    def qk_chunk(p, sc):
        """q/k projections for pair p, seq chunk sc."""
        ssl = slice(SC * sc, SC * (sc + 1))
        ps_q = ps_qk.tile([P, SC], F32, tag="psqk", name=f"psq{p}_{sc}")
        for i in range(NI):
            nc.tensor.matmul(ps_q[:], lhsT=wq[:, i, P * p:P * (p + 1)],
                             rhs=xt[:, i, ssl], start=(i == 0), stop=(i == 7))
        nc.vector.tensor_add(qT_pair[p][:, ssl], ps_q[:],
                             bq_t[:, p:p + 1].to_broadcast((P, SC)))
        ps_k = ps_qk.tile([P, SC], F32, tag="psqk", name=f"psk{p}_{sc}")
        for i in range(NI):
            nc.tensor.matmul(ps_k[:], lhsT=wk[:, i, P * p:P * (p + 1)],
                             rhs=xt[:, i, ssl], start=(i == 0), stop=(i == 7))
        nc.vector.tensor_add(kT_pair[p][:, ssl], ps_k[:],
                             bk_t[:, p:p + 1].to_broadcast((P, SC)))

    def v_chunk(sc):
        """v projection for seq chunk sc (all 8 heads), into v_aug."""
        for st in range(4):
            t = 4 * sc + st
            ps_v = ps_qk.tile([P, DG], F32, tag="psqk", name=f"psv{sc}_{st}")
            for i in range(NI):
                nc.tensor.matmul(ps_v[:], lhsT=xt[:, i, P * t:P * (t + 1)],
                                 rhs=wv[:, i, :], start=(i == 0), stop=(i == 7))
            nc.vector.tensor_add(v_aug[:, t, :, DK:P],
                                 ps_v[:].rearrange("p (h c) -> p h c", c=DK),
                                 bv_t.rearrange("p (h c) -> p h c", c=DK))

    def attention_chunk(p, j, att, slots, early_cb):
        """Causal attention for head pair p, q chunk j. Both heads row-packed
        into one wide psum; one wide exp; diag blocks first. slots maps block
        index -> filler callable; early_cb (prev chunk's normalize+AG) is
        emitted at block 2."""
        avs = [ps_av.tile([P, SC], F32, tag="av", name=f"av{p}_{j}_{h}")
               for h in range(2)]
        nkb = 4 * (j + 1)
        kbs = list(range(4 * j, nkb)) + list(range(0, 4 * j))  # diag first
        pending = []
        issued = [0]

        def issue_av(item):
            kb, qlo, et = item
            et3 = et.rearrange("p (h w) -> p h w", w=SC)
            for h in range(2):
                hh = 2 * p + h
                nc.tensor.matmul(avs[h][:, qlo:], lhsT=v_aug[:, kb, hh, :],
                                 rhs=et3[:, h, qlo:],
                                 start=(issued[0] == 0),
                                 stop=(issued[0] == nkb - 1))
            issued[0] += 1

        for n, kb in enumerate(kbs):
            r = kb - 4 * j  # >= 0 on diagonal blocks
            qlo = P * r if r >= 0 else 0
            ps_s = ps_sc.tile([P, 2 * SC], F32, tag="sc", name=f"pss{p}_{j}_{kb}")
            for h in range(2):
                hb = slice(DK * h, DK * (h + 1))
                nc.tensor.matmul(
                    ps_s[:, SC * h + qlo:SC * (h + 1)],
                    lhsT=kT_pair[p][hb, P * kb:P * (kb + 1)],
                    rhs=qT_pair[p][hb, SC * j + qlo:SC * (j + 1)],
                    start=True, stop=True)
            et = exp_pool.tile([P, 2 * SC], BF16, tag="exp", name=f"et{p}_{j}_{kb}")
            ps3 = ps_s.rearrange("p (h w) -> p h w", w=SC)
            et3 = et.rearrange("p (h w) -> p h w", w=SC)
            nc.scalar.activation(et3[:, :, qlo:], ps3[:, :, qlo:],
                                 mybir.ActivationFunctionType.Exp, scale=0.125)
            if r >= 0:
                # only the 128-wide triangle needs masking; beyond it every
                # key of this block is causally valid
                nc.vector.tensor_mul(
                    et3[:, :, qlo:qlo + P], et3[:, :, qlo:qlo + P],
                    mask128[:, 0:1, :].to_broadcast((P, 2, P)))
            pending.append((kb, qlo, et))
            while len(pending) > 3:
                issue_av(pending.pop(0))
            if early_cb is not None and n == 2:
                early_cb()
                early_cb = None
            f = slots.get(n)
            if f is not None:
                f()
        while pending:
            issue_av(pending.pop(0))
        if early_cb is not None:
            early_cb()

        def normalize():
            for h in range(2):
                sums = small.tile([1, SC], F32, tag="sums", name=f"sm{p}_{j}_{h}")
                nc.vector.reciprocal_approx_fast(sums[0:1, :], avs[h][0:1, :])
                rb = small.tile([P, SC], F32, tag="rb", name=f"rb{p}_{j}_{h}")
                nc.gpsimd.partition_broadcast(rb[:], sums[0:1, :])
                nc.vector.tensor_mul(att[h][DK:P, SC * j:SC * (j + 1)],
                                     avs[h][DK:P, :], rb[DK:P, :])
        return normalize

    # agt_all[:, i, :]: gathered attention outputs (out-proj lhsT); i = d-block
    # of the full 1024-dim attention output, i = 4*g + pair
    agt_all = const.tile([P, NI, S], BF16, name="agt_all")
    agt4 = agt_all.rearrange("p (g q) s -> p g q s", q=4)
    # out-proj partials from phase A (blocks 0,1,4,5,2,6), bf16, bo included
    part_lo = const.tile([P, NI, SC], BF16, tag="wq", name="part_lo")
    part_hi = const.tile([P, NI, SC], BF16, tag="wk", name="part_hi")

    def part_slice(qt):
        t = part_lo if qt < 8 else part_hi
        return t[:, qt % 8, :]

    def outproj_a(qt):
        """Phase A: accumulate blocks 0,1,4,5,2,6 for q-tile qt (runs as a
        filler during pair 3, by which time AG0 and AG1 have landed)."""
        ps_o = ps_qk.tile([P, DG], F32, tag="psqk", name=f"psoa{qt}")
        for n, i in enumerate([0, 1, 4, 5, 2, 6]):
            nc.tensor.matmul(ps_o[:], lhsT=agt_all[:, i, P * qt:P * (qt + 1)],
                             rhs=wo[:, i, :], start=(n == 0), stop=(n == 5))
        nc.vector.tensor_add(part_slice(qt), ps_o[:], bo_t)

    def outproj_b(qt):
        """Phase B: blocks 3,7 + phase-A partial -> out (bf16 store)."""
        ps_o = ps_qk.tile([P, DG], F32, tag="psqk", name=f"psob{qt}")
        for n, i in enumerate([3, 7]):
            nc.tensor.matmul(ps_o[:], lhsT=agt_all[:, i, P * qt:P * (qt + 1)],
                             rhs=wo[:, i, :], start=(n == 0), stop=(n == 1))
        ot = out_pool.tile([P, DG], BF16, tag="ot", name=f"ot{qt}")
        nc.vector.tensor_add(ot[:], ps_o[:], part_slice(qt))
        eng = nc.sync if qt % 2 == 0 else nc.scalar
        eng.dma_start(out[P * qt:P * (qt + 1), :], ot[:])

    def send_pair(p):
        """agin store for pairs 0-2 (full sequence)."""
        nc.sync.dma_start(agin[p, 0:DK], att_tiles[p][0][DK:P, :])
        nc.sync.dma_start(agin[p, DK:P], att_tiles[p][1][DK:P, :])

    def send_p3_chunk(j):
        """agin store + AllGather + agt reload for pair 3, chunk j."""
        csl = slice(SC * j, SC * (j + 1))
        nc.sync.dma_start(agin3[j, 0:DK], att_tiles[3][0][DK:P, csl])
        nc.sync.dma_start(agin3[j, DK:P], att_tiles[3][1][DK:P, csl])
        nc.gpsimd.collective_compute(
            "AllGather", mybir.AluOpType.bypass, replica_groups=groups,
            ins=[agin3[j][:].opt()], outs=[agout2[j][:].opt()])
        nc.sync.dma_start(agt4[:, :, 3, csl],
                          agout2[j].rearrange("g p s -> p g s"))

    # ---- prologue: QKV projections ----
    for sc in range(NSC):
        qk_chunk(0, sc)
        v_chunk(sc)

    # ---- attention, pair-pipelined ----
    att_tiles = []
    for p in range(NPAIR):
        att_tiles.append(
            [att_pool.tile([P, S], BF16, tag=f"att{h}", name=f"att{p}_{h}")
             for h in range(2)])

    norm_prev = None
    for p in range(3):
        att = att_tiles[p]
        for j in range(NSC):
            slots = {1: (lambda pp=p + 1, sc=2 * j: qk_chunk(pp, sc)),
                     3: (lambda pp=p + 1, sc=2 * j + 1: qk_chunk(pp, sc))} \
                if j < 2 else {}
            norm_j = attention_chunk(p, j, att, slots, norm_prev)
            norm_prev = norm_j
        # close out this pair: last chunk's norm, agin store, AllGather
        norm_prev()
        norm_prev = None
        send_pair(p)
        if p == 1:
            nc.gpsimd.collective_compute(
                "AllGather", mybir.AluOpType.bypass, replica_groups=groups,
                ins=[agin[0:2].opt()], outs=[agout0[:].opt()])
            for r in range(2):
                nc.sync.dma_start(agt4[:, :, r, :],
                                  agout0[:, r].rearrange("g p s -> p g s"))
        if p == 2:
            nc.gpsimd.collective_compute(
                "AllGather", mybir.AluOpType.bypass, replica_groups=groups,
                ins=[agin[2:3].opt()], outs=[agout1[:].opt()])
            nc.sync.dma_start(agt4[:, :, 2, :],
                              agout1.rearrange("g p s -> p g s"))

    # ---- pair 3: descending chunks, per-chunk AllGather, out-proj fillers ----
    att = att_tiles[3]
    # filler schedule per chunk (chunk order j=3,2,1,0):
    #   j3 (16 blocks): phase A for 11 q-tiles (delayed past AG1 issue)
    #   j2 (12 blocks): remaining phase A + phase B 12-15 (after AG of j3)
    #   j1 (8 blocks):  phase B 8-11 (after AG of j2)
    #   j0 (4 blocks):  clean; B 4-7 and 0-3 run in the epilogue
    slot_plan = {
        3: {n: (lambda qt=n - 7: outproj_a(qt)) for n in range(7, 16)},
        2: {**{n: (lambda qt=8 + n: outproj_a(qt)) for n in range(1, 8)},
            **{n: (lambda qt=4 + n: outproj_b(qt)) for n in range(8, 12)}},
        1: {n: (lambda qt=4 + n: outproj_b(qt)) for n in range(4, 8)},
        0: {},
    }
    for jj, j in enumerate([3, 2, 1, 0]):
        def early(nj=norm_prev, sj=j + 1 if j < 3 else None):
            if nj is not None:
                nj()
            if sj is not None:
                send_p3_chunk(sj)
        norm_prev = attention_chunk(3, j, att, slot_plan[j],
                                    early if (norm_prev or j < 3) else None)
    # epilogue: last chunk (j0) norm + AG, remaining phase B
    norm_prev()
    send_p3_chunk(0)
    for qt in range(4, 8):
        outproj_b(qt)
    for qt in range(0, 4):
        outproj_b(qt)

    for cm in reversed(ctxs):
        cm.__exit__(None, None, None)


def _prep_in_maps(x, Wq, bq, Wk, bk, Wv, bv, Wo, bo):
    bf16 = ml_dtypes.bfloat16
    in_maps = []
    k_idx = np.arange(P)[:, None]
    q_idx = np.arange(KB)[None, :]
    mask = (q_idx >= k_idx).astype(bf16)  # [128, 128]
    for c in range(8):
        b, g = divmod(c, 2)
        dsl = slice(g * DG, (g + 1) * DG)
        bc = np.empty((P, 8 + 2 * DG), dtype=np.float32)
        bc[:, 0:4] = bq[dsl].reshape(NPAIR, P).T
        bc[:, 4:8] = bk[dsl].reshape(NPAIR, P).T
        bc[:, 8:8 + DG] = np.broadcast_to(bv[dsl], (P, DG))
        bc[:, 8 + DG:] = np.broadcast_to(bo[dsl], (P, DG))
        in_maps.append({
            "xT": np.ascontiguousarray(x[b].T).astype(bf16),
            "wqT": np.ascontiguousarray(Wq[dsl].T).astype(bf16),
            "wkT": np.ascontiguousarray(Wk[dsl].T).astype(bf16),
            "wvT": np.ascontiguousarray(Wv[dsl].T).astype(bf16),
            "woT": np.ascontiguousarray(Wo[dsl].T).astype(bf16),
            "bconst": bc,
            "maskd": mask,
        })
    return in_maps


def kernel(x, Wq, bq, Wk, bk, Wv, bv, Wo, bo, _trace=False, _trace_kwargs=None):
    x, Wq, bq, Wk, bk = map(np.asarray, (x, Wq, bq, Wk, bk))
    Wv, bv, Wo, bo = map(np.asarray, (Wv, bv, Wo, bo))
    if "nc" not in _cache:
        _cache["nc"] = _build()
    nc = _cache["nc"]
    in_maps = _prep_in_maps(x, Wq, bq, Wk, bk, Wv, bv, Wo, bo)
    res = bass_utils.run_bass_kernel_spmd(
        nc, in_maps, core_ids=list(range(8)), trace=_trace,
        **(_trace_kwargs or {}))
    _cache["last_result"] = res
    out = np.empty((B, S, D), dtype=np.float32)
    for c in range(8):
        b, g = divmod(c, 2)
        out[b, :, g * DG:(g + 1) * DG] = np.asarray(
            res.results[c]["out"], dtype=np.float32)
    return out


# revision 11
# speedup vs baseline: 1.0133x; 1.0133x over previous
"""Causal multi-head attention on 8 TRN2 NeuronCores.

Sharding: core c = (batch b=c//2, head-group g=c%2). Each core computes QKV
projections + causal attention for its 8 heads over the full sequence of its
batch; 2-rank AllGathers (pair shares a batch) exchange attention outputs;
each core then runs the output projection for its half of the output columns.

Perf structure (v2):
  - DMAs consolidated into multi-dim-AP transfers and split across the two
    HW DGE queues (sync + scalar) so the PE never waits on a serialized
    descriptor chain.
  - v_aug ones/zeros pattern built by gpsimd memsets (no DMA).
  - Causal mask multiply shrunk to the 128-wide diagonal triangle (the mask
    is identical for every diagonal block).
  - Softmax normalize runs early (emitted at block 2 of the following chunk)
    straight from PSUM -- no staging copy; reciprocal on DVE, partition
    broadcast on gpsimd.
  - Last pair processes q-chunks in descending order with one AllGather per
    chunk; output projection phase A covers blocks [0,1,4,5,2,6] so phase B
    is just [3,7]+store, emitted as fillers as each AllGather lands.
  - Output stored as bf16, stores alternate between the two DMA queues.
"""

import numpy as np
import ml_dtypes

import concourse.bass as bass
import concourse.mybir as mybir
import concourse.tile as tile
from concourse import bacc
from concourse import bass_utils

BF16 = mybir.dt.bfloat16
F32 = mybir.dt.float32

B, S, D = 4, 2048, 1024
H, DK = 16, 64
HPG = 8          # heads per group (per core)
DG = HPG * DK    # 512, d-range per core
NPAIR = 4        # head pairs per core
SC = 512         # sequence chunk (matmul free dim)
NSC = S // SC    # 4
KB = 128         # key block
NKB = S // KB    # 16
P = 128
NI = D // P      # 8

_cache = {}
DEBUG = False


def _build():
    nc = bacc.Bacc("TRN2", target_bir_lowering=False, debug=False, num_devices=8)

    xT = nc.dram_tensor("xT", [D, S], BF16, kind="ExternalInput")
    wqT = nc.dram_tensor("wqT", [D, DG], BF16, kind="ExternalInput")
    wkT = nc.dram_tensor("wkT", [D, DG], BF16, kind="ExternalInput")
    wvT = nc.dram_tensor("wvT", [D, DG], BF16, kind="ExternalInput")
    woT = nc.dram_tensor("woT", [D, DG], BF16, kind="ExternalInput")
    # bconst: bq [0:4], bk [4:8], bv broadcast [8:520], bo broadcast [520:1032]
    bconst = nc.dram_tensor("bconst", [P, 8 + 2 * DG], F32, kind="ExternalInput")
    maskd = nc.dram_tensor("maskd", [P, KB], BF16, kind="ExternalInput")
    out = nc.dram_tensor("out", [S, DG], BF16, kind="ExternalOutput")

    with tile.TileContext(nc) as tc:
        _emit(nc, tc, xT, wqT, wkT, wvT, woT, bconst, maskd, out)
    nc.compile()
    return nc


def _emit(nc, tc, xT, wqT, wkT, wvT, woT, bconst_d, maskd, out):
    ctxs = []

    def pool(name, bufs, space="SBUF"):
        cm = tc.tile_pool(name=name, bufs=bufs, space=space)
        p = cm.__enter__()
        ctxs.append(cm)
        return p

    const = pool("const", 1)
    dram = pool("dram", 1, space="DRAM")
    qk_pool = pool("qk", 2)
    att_pool = pool("att", 2)
    exp_pool = pool("exp", 5)
    small = pool("small", 3)
    out_pool = pool("outp", 2)
    ps_qk = pool("ps_qk", 2, space="PSUM")
    ps_sc = pool("ps_sc", 2, space="PSUM")
    ps_av = pool("ps_av", 2, space="PSUM")

    # ---- constants / weights ----
    xt = const.tile([P, NI, S], BF16, name="xt")
    wq = const.tile([P, NI, DG], BF16, name="wq")
    wk = const.tile([P, NI, DG], BF16, name="wk")
    wv = const.tile([P, NI, DG], BF16, name="wv")
    wo = const.tile([P, NI, DG], BF16, name="wo")
    bconst = const.tile([P, 8 + 2 * DG], F32, name="bconst")
    mask128 = const.tile([P, 1, KB], BF16, name="mask128")
    # v_aug[:, t, h, :]: col 0 = ones (softmax denominator row), cols 1:64 =
    # zeros (PSUM partition alignment pad), cols 64:128 = v
    v_aug = const.tile([P, NKB, HPG, P], BF16, name="v_aug")

    # weights on the scalar HW DGE queue, x on the sync queue (first-needed
    # tiles in small transfers for latency, the rest consolidated); small
    # constants via gpsimd SW DGE so they don't queue behind the weights
    wqr = wqT.rearrange("(i p) d -> p i d", p=P)
    wkr = wkT.rearrange("(i p) d -> p i d", p=P)
    xTr = xT.rearrange("(i p) s -> p i s", p=P)
    for i in range(0, NI, 2):
        nc.scalar.dma_start(wq[:, i:i + 2, :], wqr[:, i:i + 2, :])
    for i in range(0, NI, 2):
        nc.sync.dma_start(xt[:, i:i + 2, 0:SC], xTr[:, i:i + 2, 0:SC])
    for i in range(0, NI, 4):
        nc.scalar.dma_start(wk[:, i:i + 4, :], wkr[:, i:i + 4, :])
    for sc in range(1, NSC):
        nc.sync.dma_start(xt[:, :, SC * sc:SC * (sc + 1)],
                          xTr[:, :, SC * sc:SC * (sc + 1)])
    nc.scalar.dma_start(wv[:], wvT.rearrange("(i p) d -> p i d", p=P))
    nc.scalar.dma_start(wo[:], woT.rearrange("(i p) d -> p i d", p=P))
    nc.gpsimd.dma_start(bconst[:], bconst_d[:])
    nc.gpsimd.dma_start(mask128[:], maskd[:])

    # v_aug constant pattern via gpsimd (keeps both DMA queues clear)
    for t in range(NKB):
        nc.gpsimd.memset(v_aug[:, t, :, 1:DK], 0.0)
        nc.gpsimd.memset(v_aug[:, t, :, 0:1], 1.0)

    bq_t = bconst[:, 0:4]
    bk_t = bconst[:, 4:8]
    bv_t = bconst[:, 8:8 + DG]
    bo_t = bconst[:, 8 + DG:8 + 2 * DG]

    # DRAM bounce buffers for the per-(pair, chunk) AllGathers.  One small
    # collective per chunk keeps every AllGather far ahead of its consumer;
    # the first one (pair 0 chunk 0) absorbs the cross-core launch skew
    # ~100us before anything reads gathered data.
    agin = dram.tile([NPAIR, NSC, P, SC], BF16, name="agin")
    agout = dram.tile([NPAIR, NSC, 2, P, SC], BF16, name="agout")

    groups = [[0, 1], [2, 3], [4, 5], [6, 7]]

    qT_pair = [qk_pool.tile([P, S], BF16, tag="qT", name=f"qTp{pp}")
               for pp in range(NPAIR)]
    kT_pair = [qk_pool.tile([P, S], BF16, tag="kT", name=f"kTp{pp}")
               for pp in range(NPAIR)]

    def qk_chunk(p, sc):
        """q/k projections for pair p, seq chunk sc."""
        ssl = slice(SC * sc, SC * (sc + 1))
        ps_q = ps_qk.tile([P, SC], F32, tag="psqk", name=f"psq{p}_{sc}")
        for i in range(NI):
            nc.tensor.matmul(ps_q[:], lhsT=wq[:, i, P * p:P * (p + 1)],
                             rhs=xt[:, i, ssl], start=(i == 0), stop=(i == 7))
        nc.vector.tensor_add(qT_pair[p][:, ssl], ps_q[:],
                             bq_t[:, p:p + 1].to_broadcast((P, SC)))
        ps_k = ps_qk.tile([P, SC], F32, tag="psqk", name=f"psk{p}_{sc}")
        for i in range(NI):
            nc.tensor.matmul(ps_k[:], lhsT=wk[:, i, P * p:P * (p + 1)],
                             rhs=xt[:, i, ssl], start=(i == 0), stop=(i == 7))
        nc.vector.tensor_add(kT_pair[p][:, ssl], ps_k[:],
                             bk_t[:, p:p + 1].to_broadcast((P, SC)))

    def v_chunk(sc):
        """v projection for seq chunk sc (all 8 heads), into v_aug."""
        for st in range(4):
            t = 4 * sc + st
            ps_v = ps_qk.tile([P, DG], F32, tag="psqk", name=f"psv{sc}_{st}")
            for i in range(NI):
                nc.tensor.matmul(ps_v[:], lhsT=xt[:, i, P * t:P * (t + 1)],
                                 rhs=wv[:, i, :], start=(i == 0), stop=(i == 7))
            nc.vector.tensor_add(v_aug[:, t, :, DK:P],
                                 ps_v[:].rearrange("p (h c) -> p h c", c=DK),
                                 bv_t.rearrange("p (h c) -> p h c", c=DK))

    def attention_chunk(p, j, att, slots, early_cb):
        """Causal attention for head pair p, q chunk j. Both heads row-packed
        into one wide psum; one wide exp; diag blocks first. slots maps block
        index -> filler callable; early_cb (prev chunk's normalize+AG) is
        emitted at block 2."""
        avs = [ps_av.tile([P, SC], F32, tag="av", name=f"av{p}_{j}_{h}")
               for h in range(2)]
        nkb = 4 * (j + 1)
        kbs = list(range(4 * j, nkb)) + list(range(0, 4 * j))  # diag first
        pending = []
        issued = [0]

        def issue_av(item):
            kb, qlo, et = item
            et3 = et.rearrange("p (h w) -> p h w", w=SC)
            for h in range(2):
                hh = 2 * p + h
                nc.tensor.matmul(avs[h][:, qlo:], lhsT=v_aug[:, kb, hh, :],
                                 rhs=et3[:, h, qlo:],
                                 start=(issued[0] == 0),
                                 stop=(issued[0] == nkb - 1))
            issued[0] += 1

        for n, kb in enumerate(kbs):
            r = kb - 4 * j  # >= 0 on diagonal blocks
            qlo = P * r if r >= 0 else 0
            ps_s = ps_sc.tile([P, 2 * SC], F32, tag="sc", name=f"pss{p}_{j}_{kb}")
            for h in range(2):
                hb = slice(DK * h, DK * (h + 1))
                nc.tensor.matmul(
                    ps_s[:, SC * h + qlo:SC * (h + 1)],
                    lhsT=kT_pair[p][hb, P * kb:P * (kb + 1)],
                    rhs=qT_pair[p][hb, SC * j + qlo:SC * (j + 1)],
                    start=True, stop=True)
            et = exp_pool.tile([P, 2 * SC], BF16, tag="exp", name=f"et{p}_{j}_{kb}")
            ps3 = ps_s.rearrange("p (h w) -> p h w", w=SC)
            et3 = et.rearrange("p (h w) -> p h w", w=SC)
            nc.scalar.activation(et3[:, :, qlo:], ps3[:, :, qlo:],
                                 mybir.ActivationFunctionType.Exp, scale=0.125)
            if r >= 0:
                # only the 128-wide triangle needs masking; beyond it every
                # key of this block is causally valid
                nc.vector.tensor_mul(
                    et3[:, :, qlo:qlo + P], et3[:, :, qlo:qlo + P],
                    mask128[:, 0:1, :].to_broadcast((P, 2, P)))
            pending.append((kb, qlo, et))
            while len(pending) > 3:
                issue_av(pending.pop(0))
            if early_cb is not None and n == 2:
                early_cb()
                early_cb = None
            f = slots.get(n)
            if f is not None:
                f()
        while pending:
            issue_av(pending.pop(0))
        if early_cb is not None:
            early_cb()

        def normalize():
            for h in range(2):
                sums = small.tile([1, SC], F32, tag="sums", name=f"sm{p}_{j}_{h}")
                nc.vector.reciprocal_approx_fast(sums[0:1, :], avs[h][0:1, :])
                rb = small.tile([P, SC], F32, tag="rb", name=f"rb{p}_{j}_{h}")
                nc.gpsimd.partition_broadcast(rb[:], sums[0:1, :])
                nc.vector.tensor_mul(att[h][DK:P, SC * j:SC * (j + 1)],
                                     avs[h][DK:P, :], rb[DK:P, :])
        return normalize

    # agt_all[:, i, :]: gathered attention outputs (out-proj lhsT); i = d-block
    # of the full 1024-dim attention output, i = 4*g + pair
    agt_all = const.tile([P, NI, S], BF16, name="agt_all")
    agt4 = agt_all.rearrange("p (g q) s -> p g q s", q=4)
    # out-proj partials from phase A (blocks 0,1,4,5,2,6), bf16, bo included
    part_lo = const.tile([P, NI, SC], BF16, tag="wq", name="part_lo")
    part_hi = const.tile([P, NI, SC], BF16, tag="wk", name="part_hi")

    def part_slice(qt):
        t = part_lo if qt < 8 else part_hi
        return t[:, qt % 8, :]

    def outproj_a(qt):
        """Phase A partial for q-tile qt (filler during pair 3).  q-tiles 0-7
        also fold in blocks 2,6 (pair-2 chunks 0/1 have gathered long before
        pair 3 runs), leaving their phase B as just 2 matmuls on the tail."""
        blocks = [0, 1, 4, 5, 2, 6] if qt < 8 else [0, 1, 4, 5]
        ps_o = ps_qk.tile([P, DG], F32, tag="psqk", name=f"psoa{qt}")
        for n, i in enumerate(blocks):
            nc.tensor.matmul(ps_o[:], lhsT=agt_all[:, i, P * qt:P * (qt + 1)],
                             rhs=wo[:, i, :], start=(n == 0),
                             stop=(n == len(blocks) - 1))
        nc.vector.tensor_add(part_slice(qt), ps_o[:], bo_t)

    def outproj_b(qt):
        """Phase B: remaining blocks + phase-A partial -> out (bf16 store)."""
        blocks = [3, 7] if qt < 8 else [2, 6, 3, 7]
        ps_o = ps_qk.tile([P, DG], F32, tag="psqk", name=f"psob{qt}")
        for n, i in enumerate(blocks):
            nc.tensor.matmul(ps_o[:], lhsT=agt_all[:, i, P * qt:P * (qt + 1)],
                             rhs=wo[:, i, :], start=(n == 0),
                             stop=(n == len(blocks) - 1))
        ot = out_pool.tile([P, DG], BF16, tag="ot", name=f"ot{qt}")
        nc.vector.tensor_add(ot[:], ps_o[:], part_slice(qt))
        eng = nc.sync if qt % 2 == 0 else nc.scalar
        eng.dma_start(out[P * qt:P * (qt + 1), :], ot[:])

    def send_chunk(p, j):
        """agin store + AllGather + agt reload for pair p, chunk j."""
        csl = slice(SC * j, SC * (j + 1))
        nc.sync.dma_start(agin[p, j, 0:DK], att_tiles[p][0][DK:P, csl])
        nc.sync.dma_start(agin[p, j, DK:P], att_tiles[p][1][DK:P, csl])
        nc.gpsimd.collective_compute(
            "AllGather", mybir.AluOpType.bypass, replica_groups=groups,
            ins=[agin[p, j][:].opt()], outs=[agout[p, j][:].opt()])
        nc.sync.dma_start(agt4[:, :, p, csl],
                          agout[p, j].rearrange("g p s -> p g s"))

    # ---- prologue: QKV projections ----
    for sc in range(NSC):
        qk_chunk(0, sc)
        v_chunk(sc)

    # ---- attention, pair-pipelined, per-(pair, chunk) AllGathers ----
    att_tiles = []
    for p in range(NPAIR):
        att_tiles.append(
            [att_pool.tile([P, S], BF16, tag=f"att{h}", name=f"att{p}_{h}")
             for h in range(2)])

    # pair-3 filler schedule (chunk order j=3,2,1,0):
    #   j3 (16 blocks): phase A q-tiles 0-10
    #   j2 (12 blocks): phase A 11-15, then phase B 12-15 (pair-3 chunk-3
    #                   AllGather was issued at block 2 of this chunk)
    #   j1 (8 blocks):  phase B 8-11
    #   j0 (4 blocks):  clean; B 4-7 and 0-3 run in the epilogue
    p3_slots = {
        3: {n: (lambda qt=n - 5: outproj_a(qt)) for n in range(5, 16)},
        2: {n: (lambda qt=10 + n: outproj_a(qt)) for n in range(1, 6)},
        1: {n: (lambda qt=12 + n: outproj_b(qt)) for n in range(0, 4)},
        0: {n: (lambda qt=8 + n: outproj_b(qt)) for n in range(0, 4)},
    }

    norm_prev = None
    send_prev = None
    for p in range(NPAIR):
        jorder = [0, 1, 2, 3] if p < 3 else [3, 2, 1, 0]
        for j in jorder:
            if p < 3:
                slots = {1: (lambda pp=p + 1, sc=2 * j: qk_chunk(pp, sc)),
                         3: (lambda pp=p + 1, sc=2 * j + 1: qk_chunk(pp, sc))} \
                    if j < 2 else {}
            else:
                slots = p3_slots[j]
            if norm_prev is not None:
                def early(nj=norm_prev, sp=send_prev):
                    nj()
                    send_chunk(*sp)
            else:
                early = None
            norm_prev = attention_chunk(p, j, att_tiles[p], slots, early)
            send_prev = (p, j)
    # epilogue: last chunk (pair 3, j0) norm + AG, remaining phase B
    norm_prev()
    send_chunk(3, 0)
    for qt in range(4, 8):
        outproj_b(qt)
    for qt in range(0, 4):
        outproj_b(qt)

    for cm in reversed(ctxs):
        cm.__exit__(None, None, None)


def _prep_in_maps(x, Wq, bq, Wk, bk, Wv, bv, Wo, bo):
    bf16 = ml_dtypes.bfloat16
    in_maps = []
    k_idx = np.arange(P)[:, None]
    q_idx = np.arange(KB)[None, :]
    mask = (q_idx >= k_idx).astype(bf16)  # [128, 128]
    for c in range(8):
        b, g = divmod(c, 2)
        dsl = slice(g * DG, (g + 1) * DG)
        bc = np.empty((P, 8 + 2 * DG), dtype=np.float32)
        bc[:, 0:4] = bq[dsl].reshape(NPAIR, P).T
        bc[:, 4:8] = bk[dsl].reshape(NPAIR, P).T
        bc[:, 8:8 + DG] = np.broadcast_to(bv[dsl], (P, DG))
        bc[:, 8 + DG:] = np.broadcast_to(bo[dsl], (P, DG))
        in_maps.append({
            "xT": np.ascontiguousarray(x[b].T).astype(bf16),
            "wqT": np.ascontiguousarray(Wq[dsl].T).astype(bf16),
            "wkT": np.ascontiguousarray(Wk[dsl].T).astype(bf16),
            "wvT": np.ascontiguousarray(Wv[dsl].T).astype(bf16),
            "woT": np.ascontiguousarray(Wo[dsl].T).astype(bf16),
            "bconst": bc,
            "maskd": mask,
        })
    return in_maps


def kernel(x, Wq, bq, Wk, bk, Wv, bv, Wo, bo, _trace=False, _trace_kwargs=None):
    x, Wq, bq, Wk, bk = map(np.asarray, (x, Wq, bq, Wk, bk))
    Wv, bv, Wo, bo = map(np.asarray, (Wv, bv, Wo, bo))
    if "nc" not in _cache:
        _cache["nc"] = _build()
    nc = _cache["nc"]
    in_maps = _prep_in_maps(x, Wq, bq, Wk, bk, Wv, bv, Wo, bo)
    res = bass_utils.run_bass_kernel_spmd(
        nc, in_maps, core_ids=list(range(8)), trace=_trace,
        **(_trace_kwargs or {}))
    _cache["last_result"] = res
    out = np.empty((B, S, D), dtype=np.float32)
    for c in range(8):
        b, g = divmod(c, 2)
        out[b, :, g * DG:(g + 1) * DG] = np.asarray(
            res.results[c]["out"], dtype=np.float32)
    return out


# revision 13
# speedup vs baseline: 1.0739x; 1.0598x over previous
"""Causal multi-head attention on 8 TRN2 NeuronCores.

Sharding: core c = (batch b=c//2, head-group g=c%2). Each core computes QKV
projections + causal attention for its 8 heads over the full sequence of its
batch; 2-rank AllGathers (pair shares a batch) exchange attention outputs;
each core then runs the output projection for its half of the output columns.

Perf structure (v2):
  - DMAs consolidated into multi-dim-AP transfers and split across the two
    HW DGE queues (sync + scalar) so the PE never waits on a serialized
    descriptor chain.
  - v_aug ones/zeros pattern built by gpsimd memsets (no DMA).
  - Causal mask multiply shrunk to the 128-wide diagonal triangle (the mask
    is identical for every diagonal block).
  - Softmax normalize runs early (emitted at block 2 of the following chunk)
    straight from PSUM -- no staging copy; reciprocal on DVE, partition
    broadcast on gpsimd.
  - Last pair processes q-chunks in descending order with one AllGather per
    chunk; output projection phase A covers blocks [0,1,4,5,2,6] so phase B
    is just [3,7]+store, emitted as fillers as each AllGather lands.
  - Output stored as bf16, stores alternate between the two DMA queues.
"""

import numpy as np
import ml_dtypes

import concourse.bass as bass
import concourse.mybir as mybir
import concourse.tile as tile
from concourse import bacc
from concourse import bass_utils

BF16 = mybir.dt.bfloat16
F32 = mybir.dt.float32

B, S, D = 4, 2048, 1024
H, DK = 16, 64
HPG = 8          # heads per group (per core)
DG = HPG * DK    # 512, d-range per core
NPAIR = 4        # head pairs per core
SC = 512         # sequence chunk (matmul free dim)
NSC = S // SC    # 4
KB = 128         # key block
NKB = S // KB    # 16
P = 128
NI = D // P      # 8

_cache = {}
DEBUG = False


def _build():
    nc = bacc.Bacc("TRN2", target_bir_lowering=False, debug=False, num_devices=8)

    xT = nc.dram_tensor("xT", [D, S], BF16, kind="ExternalInput")
    wqT = nc.dram_tensor("wqT", [D, DG], BF16, kind="ExternalInput")
    wkT = nc.dram_tensor("wkT", [D, DG], BF16, kind="ExternalInput")
    wvT = nc.dram_tensor("wvT", [D, DG], BF16, kind="ExternalInput")
    woT = nc.dram_tensor("woT", [D, DG], BF16, kind="ExternalInput")
    # bconst: bq [0:4], bk [4:8], bv broadcast [8:520], bo broadcast [520:1032]
    bconst = nc.dram_tensor("bconst", [P, 8 + 2 * DG], F32, kind="ExternalInput")
    maskd = nc.dram_tensor("maskd", [P, KB], BF16, kind="ExternalInput")
    out = nc.dram_tensor("out", [S, DG], BF16, kind="ExternalOutput")

    with tile.TileContext(nc) as tc:
        _emit(nc, tc, xT, wqT, wkT, wvT, woT, bconst, maskd, out)
    nc.compile()
    return nc


def _emit(nc, tc, xT, wqT, wkT, wvT, woT, bconst_d, maskd, out):
    ctxs = []

    def pool(name, bufs, space="SBUF"):
        cm = tc.tile_pool(name=name, bufs=bufs, space=space)
        p = cm.__enter__()
        ctxs.append(cm)
        return p

    const = pool("const", 1)
    dram = pool("dram", 1, space="DRAM")
    qk_pool = pool("qk", 2)
    att_pool = pool("att", 2)
    exp_pool = pool("exp", 5)
    small = pool("small", 3)
    out_pool = pool("outp", 2)
    ps_qk = pool("ps_qk", 2, space="PSUM")
    ps_sc = pool("ps_sc", 2, space="PSUM")
    ps_av = pool("ps_av", 2, space="PSUM")

    # ---- constants / weights ----
    xt = const.tile([P, NI, S], BF16, name="xt")
    wq = const.tile([P, NI, DG], BF16, name="wq")
    wk = const.tile([P, NI, DG], BF16, name="wk")
    wv = const.tile([P, NI, DG], BF16, name="wv")
    wo = const.tile([P, NI, DG], BF16, name="wo")
    bconst = const.tile([P, 8 + 2 * DG], F32, name="bconst")
    mask128 = const.tile([P, 1, KB], BF16, name="mask128")
    # v_aug[:, t, h, :]: col 0 = ones (softmax denominator row), cols 1:64 =
    # zeros (PSUM partition alignment pad), cols 64:128 = v
    v_aug = const.tile([P, NKB, HPG, P], BF16, name="v_aug")

    # weights on the scalar HW DGE queue, x on the sync queue (first-needed
    # tiles in small transfers for latency, the rest consolidated); small
    # constants via gpsimd SW DGE so they don't queue behind the weights
    wqr = wqT.rearrange("(i p) d -> p i d", p=P)
    wkr = wkT.rearrange("(i p) d -> p i d", p=P)
    xTr = xT.rearrange("(i p) s -> p i s", p=P)
    for i in range(0, NI, 2):
        nc.scalar.dma_start(wq[:, i:i + 2, :], wqr[:, i:i + 2, :])
    for i in range(0, NI, 2):
        nc.sync.dma_start(xt[:, i:i + 2, 0:SC], xTr[:, i:i + 2, 0:SC])
    for i in range(0, NI, 4):
        nc.scalar.dma_start(wk[:, i:i + 4, :], wkr[:, i:i + 4, :])
    nc.sync.dma_start(wv[:], wvT.rearrange("(i p) d -> p i d", p=P))
    for sc in range(1, NSC):
        nc.sync.dma_start(xt[:, :, SC * sc:SC * (sc + 1)],
                          xTr[:, :, SC * sc:SC * (sc + 1)])
    nc.scalar.dma_start(wo[:], woT.rearrange("(i p) d -> p i d", p=P))
    nc.gpsimd.dma_start(bconst[:], bconst_d[:])
    nc.gpsimd.dma_start(mask128[:], maskd[:])

    bq_t = bconst[:, 0:4]
    bk_t = bconst[:, 4:8]
    bv_t = bconst[:, 8:8 + DG]
    bo_t = bconst[:, 8 + DG:8 + 2 * DG]

    # DRAM bounce buffers for the per-(pair, chunk) AllGathers.  One small
    # collective per chunk keeps every AllGather far ahead of its consumer;
    # the first one (pair 0 chunk 0) absorbs the cross-core launch skew
    # ~100us before anything reads gathered data.
    agin = dram.tile([NPAIR, NSC, P, SC], BF16, name="agin")
    agout = dram.tile([NPAIR, NSC, 2, P, SC], BF16, name="agout")
    dummy_in = dram.tile([P, 16], BF16, name="dummy_in")
    dummy_out = dram.tile([2, P, 16], BF16, name="dummy_out")

    groups = [[0, 1], [2, 3], [4, 5], [6, 7]]

    # Tiny dummy AllGather issued at kernel start: the collective stream runs
    # in order and each trigger waits for the previous collective, so the
    # FIRST collective absorbs the cross-core launch skew (tens of us).  Fire
    # it here, where nothing depends on the gpsimd queue for a long time, so
    # every real AllGather later completes in a few us.
    nc.gpsimd.collective_compute(
        "AllGather", mybir.AluOpType.bypass, replica_groups=groups,
        ins=[dummy_in[:].opt()], outs=[dummy_out[:].opt()])

    # v_aug constant pattern via gpsimd (keeps both DMA queues clear)
    for t in range(NKB):
        nc.gpsimd.memset(v_aug[:, t, :, 1:DK], 0.0)
        nc.gpsimd.memset(v_aug[:, t, :, 0:1], 1.0)

    qT_pair = [qk_pool.tile([P, S], BF16, tag="qT", name=f"qTp{pp}")
               for pp in range(NPAIR)]
    kT_pair = [qk_pool.tile([P, S], BF16, tag="kT", name=f"kTp{pp}")
               for pp in range(NPAIR)]

    def qk_chunk(p, sc):
        """q/k projections for pair p, seq chunk sc."""
        ssl = slice(SC * sc, SC * (sc + 1))
        ps_q = ps_qk.tile([P, SC], F32, tag="psqk", name=f"psq{p}_{sc}")
        for i in range(NI):
            nc.tensor.matmul(ps_q[:], lhsT=wq[:, i, P * p:P * (p + 1)],
                             rhs=xt[:, i, ssl], start=(i == 0), stop=(i == 7))
        nc.vector.tensor_add(qT_pair[p][:, ssl], ps_q[:],
                             bq_t[:, p:p + 1].to_broadcast((P, SC)))
        ps_k = ps_qk.tile([P, SC], F32, tag="psqk", name=f"psk{p}_{sc}")
        for i in range(NI):
            nc.tensor.matmul(ps_k[:], lhsT=wk[:, i, P * p:P * (p + 1)],
                             rhs=xt[:, i, ssl], start=(i == 0), stop=(i == 7))
        nc.vector.tensor_add(kT_pair[p][:, ssl], ps_k[:],
                             bk_t[:, p:p + 1].to_broadcast((P, SC)))

    def v_chunk(sc):
        """v projection for seq chunk sc (all 8 heads), into v_aug."""
        for st in range(4):
            t = 4 * sc + st
            ps_v = ps_qk.tile([P, DG], F32, tag="psqk", name=f"psv{sc}_{st}")
            for i in range(NI):
                nc.tensor.matmul(ps_v[:], lhsT=xt[:, i, P * t:P * (t + 1)],
                                 rhs=wv[:, i, :], start=(i == 0), stop=(i == 7))
            nc.vector.tensor_add(v_aug[:, t, :, DK:P],
                                 ps_v[:].rearrange("p (h c) -> p h c", c=DK),
                                 bv_t.rearrange("p (h c) -> p h c", c=DK))

    def attention_chunk(p, j, att, slots, early_cb):
        """Causal attention for head pair p, q chunk j. Both heads row-packed
        into one wide psum; one wide exp; diag blocks first. slots maps block
        index -> filler callable; early_cb (prev chunk's normalize+AG) is
        emitted at block 2."""
        avs = [ps_av.tile([P, SC], F32, tag="av", name=f"av{p}_{j}_{h}")
               for h in range(2)]
        nkb = 4 * (j + 1)
        kbs = list(range(4 * j, nkb)) + list(range(0, 4 * j))  # diag first
        pending = []
        issued = [0]

        def issue_av(item):
            kb, qlo, et = item
            et3 = et.rearrange("p (h w) -> p h w", w=SC)
            for h in range(2):
                hh = 2 * p + h
                nc.tensor.matmul(avs[h][:, qlo:], lhsT=v_aug[:, kb, hh, :],
                                 rhs=et3[:, h, qlo:],
                                 start=(issued[0] == 0),
                                 stop=(issued[0] == nkb - 1))
            issued[0] += 1

        for n, kb in enumerate(kbs):
            r = kb - 4 * j  # >= 0 on diagonal blocks
            qlo = P * r if r >= 0 else 0
            ps_s = ps_sc.tile([P, 2 * SC], F32, tag="sc", name=f"pss{p}_{j}_{kb}")
            for h in range(2):
                hb = slice(DK * h, DK * (h + 1))
                nc.tensor.matmul(
                    ps_s[:, SC * h + qlo:SC * (h + 1)],
                    lhsT=kT_pair[p][hb, P * kb:P * (kb + 1)],
                    rhs=qT_pair[p][hb, SC * j + qlo:SC * (j + 1)],
                    start=True, stop=True)
            et = exp_pool.tile([P, 2 * SC], BF16, tag="exp", name=f"et{p}_{j}_{kb}")
            ps3 = ps_s.rearrange("p (h w) -> p h w", w=SC)
            et3 = et.rearrange("p (h w) -> p h w", w=SC)
            nc.scalar.activation(et3[:, :, qlo:], ps3[:, :, qlo:],
                                 mybir.ActivationFunctionType.Exp, scale=0.125)
            if r >= 0:
                # only the 128-wide triangle needs masking; beyond it every
                # key of this block is causally valid
                nc.vector.tensor_mul(
                    et3[:, :, qlo:qlo + P], et3[:, :, qlo:qlo + P],
                    mask128[:, 0:1, :].to_broadcast((P, 2, P)))
            pending.append((kb, qlo, et))
            while len(pending) > 3:
                issue_av(pending.pop(0))
            if early_cb is not None and n == 2:
                early_cb()
                early_cb = None
            f = slots.get(n)
            if f is not None:
                f()
        while pending:
            issue_av(pending.pop(0))
        if early_cb is not None:
            early_cb()

        def normalize():
            for h in range(2):
                sums = small.tile([1, SC], F32, tag="sums", name=f"sm{p}_{j}_{h}")
                nc.vector.reciprocal_approx_fast(sums[0:1, :], avs[h][0:1, :])
                rb = small.tile([P, SC], F32, tag="rb", name=f"rb{p}_{j}_{h}")
                nc.gpsimd.partition_broadcast(rb[:], sums[0:1, :])
                nc.vector.tensor_mul(att[h][DK:P, SC * j:SC * (j + 1)],
                                     avs[h][DK:P, :], rb[DK:P, :])
        return normalize

    # agt_all[:, i, :]: gathered attention outputs (out-proj lhsT); i = d-block
    # of the full 1024-dim attention output, i = 4*g + pair
    agt_all = const.tile([P, NI, S], BF16, name="agt_all")
    agt4 = agt_all.rearrange("p (g q) s -> p g q s", q=4)
    # out-proj partials from phase A (blocks 0,1,4,5,2,6), bf16, bo included
    part_lo = const.tile([P, NI, SC], BF16, tag="wq", name="part_lo")
    part_hi = const.tile([P, NI, SC], BF16, tag="wk", name="part_hi")

    def part_slice(qt):
        t = part_lo if qt < 8 else part_hi
        return t[:, qt % 8, :]

    def outproj_a(qt):
        """Phase A partial for q-tile qt (filler during pair 3).  q-tiles 0-7
        also fold in blocks 2,6 (pair-2 chunks 0/1 have gathered long before
        pair 3 runs), leaving their phase B as just 2 matmuls on the tail."""
        blocks = [0, 1, 4, 5, 2, 6] if qt < 8 else [0, 1, 4, 5]
        ps_o = ps_qk.tile([P, DG], F32, tag="psqk", name=f"psoa{qt}")
        for n, i in enumerate(blocks):
            nc.tensor.matmul(ps_o[:], lhsT=agt_all[:, i, P * qt:P * (qt + 1)],
                             rhs=wo[:, i, :], start=(n == 0),
                             stop=(n == len(blocks) - 1))
        nc.vector.tensor_add(part_slice(qt), ps_o[:], bo_t)

    def outproj_b(qt):
        """Phase B: remaining blocks + phase-A partial -> out (bf16 store)."""
        blocks = [3, 7] if qt < 8 else [2, 6, 3, 7]
        ps_o = ps_qk.tile([P, DG], F32, tag="psqk", name=f"psob{qt}")
        for n, i in enumerate(blocks):
            nc.tensor.matmul(ps_o[:], lhsT=agt_all[:, i, P * qt:P * (qt + 1)],
                             rhs=wo[:, i, :], start=(n == 0),
                             stop=(n == len(blocks) - 1))
        ot = out_pool.tile([P, DG], BF16, tag="ot", name=f"ot{qt}")
        nc.vector.tensor_add(ot[:], ps_o[:], part_slice(qt))
        eng = nc.sync if qt % 2 == 0 else nc.scalar
        eng.dma_start(out[P * qt:P * (qt + 1), :], ot[:])

    def send_chunk(p, j):
        """agin store + AllGather + agt reload for pair p, chunk j."""
        csl = slice(SC * j, SC * (j + 1))
        nc.sync.dma_start(agin[p, j, 0:DK], att_tiles[p][0][DK:P, csl])
        nc.sync.dma_start(agin[p, j, DK:P], att_tiles[p][1][DK:P, csl])
        nc.gpsimd.collective_compute(
            "AllGather", mybir.AluOpType.bypass, replica_groups=groups,
            ins=[agin[p, j][:].opt()], outs=[agout[p, j][:].opt()])
        nc.sync.dma_start(agt4[:, :, p, csl],
                          agout[p, j].rearrange("g p s -> p g s"))

    # ---- prologue: QKV projections ----
    for sc in range(NSC):
        qk_chunk(0, sc)
        v_chunk(sc)

    # ---- attention, pair-pipelined, per-(pair, chunk) AllGathers ----
    att_tiles = []
    for p in range(NPAIR):
        att_tiles.append(
            [att_pool.tile([P, S], BF16, tag=f"att{h}", name=f"att{p}_{h}")
             for h in range(2)])

    # pair-3 filler schedule (chunk order j=3,2,1,0):
    #   j3 (16 blocks): phase A q-tiles 0-10
    #   j2 (12 blocks): phase A 11-15, then phase B 12-15 (pair-3 chunk-3
    #                   AllGather was issued at block 2 of this chunk)
    #   j1 (8 blocks):  phase B 8-11
    #   j0 (4 blocks):  clean; B 4-7 and 0-3 run in the epilogue
    p3_slots = {
        3: {n: (lambda qt=n - 5: outproj_a(qt)) for n in range(5, 16)},
        2: {n: (lambda qt=10 + n: outproj_a(qt)) for n in range(1, 6)},
        1: {n: (lambda qt=12 + n: outproj_b(qt)) for n in range(0, 4)},
        0: {n: (lambda qt=8 + n: outproj_b(qt)) for n in range(0, 4)},
    }

    norm_prev = None
    send_prev = None
    for p in range(NPAIR):
        jorder = [0, 1, 2, 3] if p < 3 else [3, 2, 1, 0]
        for j in jorder:
            if p < 3:
                slots = {1: (lambda pp=p + 1, sc=2 * j: qk_chunk(pp, sc)),
                         3: (lambda pp=p + 1, sc=2 * j + 1: qk_chunk(pp, sc))} \
                    if j < 2 else {}
            else:
                slots = p3_slots[j]
            if norm_prev is not None:
                def early(nj=norm_prev, sp=send_prev):
                    nj()
                    send_chunk(*sp)
            else:
                early = None
            norm_prev = attention_chunk(p, j, att_tiles[p], slots, early)
            send_prev = (p, j)
    # epilogue: last chunk (pair 3, j0) norm + AG, remaining phase B
    norm_prev()
    send_chunk(3, 0)
    for qt in range(4, 8):
        outproj_b(qt)
    for qt in range(0, 4):
        outproj_b(qt)

    for cm in reversed(ctxs):
        cm.__exit__(None, None, None)


def _prep_in_maps(x, Wq, bq, Wk, bk, Wv, bv, Wo, bo):
    bf16 = ml_dtypes.bfloat16
    in_maps = []
    k_idx = np.arange(P)[:, None]
    q_idx = np.arange(KB)[None, :]
    mask = (q_idx >= k_idx).astype(bf16)  # [128, 128]
    for c in range(8):
        b, g = divmod(c, 2)
        dsl = slice(g * DG, (g + 1) * DG)
        bc = np.empty((P, 8 + 2 * DG), dtype=np.float32)
        bc[:, 0:4] = bq[dsl].reshape(NPAIR, P).T
        bc[:, 4:8] = bk[dsl].reshape(NPAIR, P).T
        bc[:, 8:8 + DG] = np.broadcast_to(bv[dsl], (P, DG))
        bc[:, 8 + DG:] = np.broadcast_to(bo[dsl], (P, DG))
        in_maps.append({
            "xT": np.ascontiguousarray(x[b].T).astype(bf16),
            "wqT": np.ascontiguousarray(Wq[dsl].T).astype(bf16),
            "wkT": np.ascontiguousarray(Wk[dsl].T).astype(bf16),
            "wvT": np.ascontiguousarray(Wv[dsl].T).astype(bf16),
            "woT": np.ascontiguousarray(Wo[dsl].T).astype(bf16),
            "bconst": bc,
            "maskd": mask,
        })
    return in_maps


def kernel(x, Wq, bq, Wk, bk, Wv, bv, Wo, bo, _trace=False, _trace_kwargs=None):
    x, Wq, bq, Wk, bk = map(np.asarray, (x, Wq, bq, Wk, bk))
    Wv, bv, Wo, bo = map(np.asarray, (Wv, bv, Wo, bo))
    if "nc" not in _cache:
        _cache["nc"] = _build()
    nc = _cache["nc"]
    in_maps = _prep_in_maps(x, Wq, bq, Wk, bk, Wv, bv, Wo, bo)
    res = bass_utils.run_bass_kernel_spmd(
        nc, in_maps, core_ids=list(range(8)), trace=_trace,
        **(_trace_kwargs or {}))
    _cache["last_result"] = res
    out = np.empty((B, S, D), dtype=np.float32)
    for c in range(8):
        b, g = divmod(c, 2)
        out[b, :, g * DG:(g + 1) * DG] = np.asarray(
            res.results[c]["out"], dtype=np.float32)
    return out


# revision 19
# speedup vs baseline: 1.0941x; 1.0188x over previous
"""Causal multi-head attention on 8 TRN2 NeuronCores.

Sharding: core c = (batch b=c//2, head-group g=c%2). Each core computes QKV
projections + causal attention for its 8 heads over the full sequence of its
batch; 2-rank AllGathers (pair shares a batch) exchange attention outputs;
each core then runs the output projection for its half of the output columns.

Perf structure (v2):
  - DMAs consolidated into multi-dim-AP transfers and split across the two
    HW DGE queues (sync + scalar) so the PE never waits on a serialized
    descriptor chain.
  - v_aug ones/zeros pattern built by gpsimd memsets (no DMA).
  - Causal mask multiply shrunk to the 128-wide diagonal triangle (the mask
    is identical for every diagonal block).
  - Softmax normalize runs early (emitted at block 2 of the following chunk)
    straight from PSUM -- no staging copy; reciprocal on DVE, partition
    broadcast on gpsimd.
  - Last pair processes q-chunks in descending order with one AllGather per
    chunk; output projection phase A covers blocks [0,1,4,5,2,6] so phase B
    is just [3,7]+store, emitted as fillers as each AllGather lands.
  - Output stored as bf16, stores alternate between the two DMA queues.
"""

import numpy as np
import ml_dtypes

import concourse.bass as bass
import concourse.mybir as mybir
import concourse.tile as tile
from concourse import bacc
from concourse import bass_utils

BF16 = mybir.dt.bfloat16
F32 = mybir.dt.float32

B, S, D = 4, 2048, 1024
H, DK = 16, 64
HPG = 8          # heads per group (per core)
DG = HPG * DK    # 512, d-range per core
NPAIR = 4        # head pairs per core
SC = 512         # sequence chunk (matmul free dim)
NSC = S // SC    # 4
KB = 128         # key block
NKB = S // KB    # 16
P = 128
NI = D // P      # 8

_cache = {}
DEBUG = False


def _build():
    nc = bacc.Bacc("TRN2", target_bir_lowering=False, debug=False, num_devices=8)

    xT = nc.dram_tensor("xT", [D, S], BF16, kind="ExternalInput")
    wqT = nc.dram_tensor("wqT", [D, DG], BF16, kind="ExternalInput")
    wkT = nc.dram_tensor("wkT", [D, DG], BF16, kind="ExternalInput")
    wvT = nc.dram_tensor("wvT", [D, DG], BF16, kind="ExternalInput")
    woT = nc.dram_tensor("woT", [D, DG], BF16, kind="ExternalInput")
    # bconst: bq [0:4], bk [4:8], bv broadcast [8:520], bo broadcast [520:1032]
    bconst = nc.dram_tensor("bconst", [P, 8 + 2 * DG], F32, kind="ExternalInput")
    maskd = nc.dram_tensor("maskd", [P, KB], BF16, kind="ExternalInput")
    out = nc.dram_tensor("out", [S, DG], BF16, kind="ExternalOutput")

    with tile.TileContext(nc) as tc:
        _emit(nc, tc, xT, wqT, wkT, wvT, woT, bconst, maskd, out)
    nc.compile()
    return nc


def _emit(nc, tc, xT, wqT, wkT, wvT, woT, bconst_d, maskd, out):
    ctxs = []

    def pool(name, bufs, space="SBUF"):
        cm = tc.tile_pool(name=name, bufs=bufs, space=space)
        p = cm.__enter__()
        ctxs.append(cm)
        return p

    const = pool("const", 1)
    dram = pool("dram", 1, space="DRAM")
    qk_pool = pool("qk", 2)
    att_pool = pool("att", 2)
    exp_pool = pool("exp", 5)
    small = pool("small", 3)
    out_pool = pool("outp", 2)
    ps_qk = pool("ps_qk", 2, space="PSUM")
    ps_sc = pool("ps_sc", 2, space="PSUM")
    ps_av = pool("ps_av", 2, space="PSUM")

    # ---- constants / weights ----
    xt = const.tile([P, NI, S], BF16, name="xt")
    wq = const.tile([P, NI, DG], BF16, name="wq")
    wk = const.tile([P, NI, DG], BF16, name="wk")
    wv = const.tile([P, NI, DG], BF16, name="wv")
    wo = const.tile([P, NI, DG], BF16, name="wo")
    bconst = const.tile([P, 8 + 2 * DG], F32, name="bconst")
    mask128 = const.tile([P, 1, KB], BF16, name="mask128")
    # v_aug[:, t, h, :]: col 0 = ones (softmax denominator row), cols 1:64 =
    # zeros (PSUM partition alignment pad), cols 64:128 = v
    v_aug = const.tile([P, NKB, HPG, P], BF16, name="v_aug")

    # weights on the scalar HW DGE queue, x on the sync queue (first-needed
    # tiles in small transfers for latency, the rest consolidated); small
    # constants via gpsimd SW DGE so they don't queue behind the weights
    wqr = wqT.rearrange("(i p) d -> p i d", p=P)
    wkr = wkT.rearrange("(i p) d -> p i d", p=P)
    xTr = xT.rearrange("(i p) s -> p i s", p=P)
    for i in range(0, NI, 2):
        nc.scalar.dma_start(wq[:, i:i + 2, :], wqr[:, i:i + 2, :])
    for i in range(0, NI, 2):
        nc.sync.dma_start(xt[:, i:i + 2, 0:SC], xTr[:, i:i + 2, 0:SC])
    for i in range(0, NI, 4):
        nc.scalar.dma_start(wk[:, i:i + 4, :], wkr[:, i:i + 4, :])
    nc.sync.dma_start(xt[:, :, SC:2 * SC], xTr[:, :, SC:2 * SC])
    nc.sync.dma_start(wv[:], wvT.rearrange("(i p) d -> p i d", p=P))
    for sc in range(2, NSC):
        nc.sync.dma_start(xt[:, :, SC * sc:SC * (sc + 1)],
                          xTr[:, :, SC * sc:SC * (sc + 1)])
    nc.scalar.dma_start(wo[:], woT.rearrange("(i p) d -> p i d", p=P))
    nc.gpsimd.dma_start(bconst[:], bconst_d[:])
    nc.gpsimd.dma_start(mask128[:], maskd[:])

    bq_t = bconst[:, 0:4]
    bk_t = bconst[:, 4:8]
    bv_t = bconst[:, 8:8 + DG]
    bo_t = bconst[:, 8 + DG:8 + 2 * DG]

    # DRAM bounce buffers for the per-(pair, chunk) AllGathers.  One small
    # collective per chunk keeps every AllGather far ahead of its consumer;
    # the first one (pair 0 chunk 0) absorbs the cross-core launch skew
    # ~100us before anything reads gathered data.
    agin = dram.tile([NPAIR, NSC, P, SC], BF16, name="agin")
    agout = dram.tile([NPAIR, NSC, 2, P, SC], BF16, name="agout")
    dummy_in = dram.tile([P, 16], BF16, name="dummy_in")
    dummy_out = dram.tile([2, P, 16], BF16, name="dummy_out")

    groups = [[0, 1], [2, 3], [4, 5], [6, 7]]

    # Tiny dummy AllGather issued at kernel start: the collective stream runs
    # in order and each trigger waits for the previous collective, so the
    # FIRST collective absorbs the cross-core launch skew (tens of us).  Fire
    # it here, where nothing depends on the gpsimd queue for a long time, so
    # every real AllGather later completes in a few us.
    nc.gpsimd.collective_compute(
        "AllGather", mybir.AluOpType.bypass, replica_groups=groups,
        ins=[dummy_in[:].opt()], outs=[dummy_out[:].opt()])

    # v_aug constant pattern via gpsimd (keeps both DMA queues clear)
    for t in range(NKB):
        nc.gpsimd.memset(v_aug[:, t, :, 1:DK], 0.0)
        nc.gpsimd.memset(v_aug[:, t, :, 0:1], 1.0)

    qT_pair = [qk_pool.tile([P, S], BF16, tag="qT", name=f"qTp{pp}")
               for pp in range(NPAIR)]
    kT_pair = [qk_pool.tile([P, S], BF16, tag="kT", name=f"kTp{pp}")
               for pp in range(NPAIR)]

    def qk_chunk(p, sc):
        """q/k projections for pair p, seq chunk sc."""
        ssl = slice(SC * sc, SC * (sc + 1))
        ps_q = ps_qk.tile([P, SC], F32, tag="psqk", name=f"psq{p}_{sc}")
        for i in range(NI):
            nc.tensor.matmul(ps_q[:], lhsT=wq[:, i, P * p:P * (p + 1)],
                             rhs=xt[:, i, ssl], start=(i == 0), stop=(i == 7))
        nc.vector.tensor_add(qT_pair[p][:, ssl], ps_q[:],
                             bq_t[:, p:p + 1].to_broadcast((P, SC)))
        ps_k = ps_qk.tile([P, SC], F32, tag="psqk", name=f"psk{p}_{sc}")
        for i in range(NI):
            nc.tensor.matmul(ps_k[:], lhsT=wk[:, i, P * p:P * (p + 1)],
                             rhs=xt[:, i, ssl], start=(i == 0), stop=(i == 7))
        nc.vector.tensor_add(kT_pair[p][:, ssl], ps_k[:],
                             bk_t[:, p:p + 1].to_broadcast((P, SC)))

    def v_chunk(sc):
        """v projection for seq chunk sc (all 8 heads), into v_aug."""
        for st in range(4):
            t = 4 * sc + st
            ps_v = ps_qk.tile([P, DG], F32, tag="psqk", name=f"psv{sc}_{st}")
            for i in range(NI):
                nc.tensor.matmul(ps_v[:], lhsT=xt[:, i, P * t:P * (t + 1)],
                                 rhs=wv[:, i, :], start=(i == 0), stop=(i == 7))
            nc.vector.tensor_add(v_aug[:, t, :, DK:P],
                                 ps_v[:].rearrange("p (h c) -> p h c", c=DK),
                                 bv_t.rearrange("p (h c) -> p h c", c=DK))

    def attention_chunk(p, j, att, slots, early_cb):
        """Causal attention for head pair p, q chunk j. Both heads row-packed
        into one wide psum; one wide exp; diag blocks first. slots maps block
        index -> filler callable; early_cb (prev chunk's normalize+AG) is
        emitted at block 2."""
        avs = [ps_av.tile([P, SC], F32, tag="av", name=f"av{p}_{j}_{h}")
               for h in range(2)]
        nkb = 4 * (j + 1)
        kbs = list(range(4 * j, nkb)) + list(range(0, 4 * j))  # diag first
        pending = []
        issued = [0]

        def issue_av(item):
            kb, qlo, et = item
            et3 = et.rearrange("p (h w) -> p h w", w=SC)
            for h in range(2):
                hh = 2 * p + h
                nc.tensor.matmul(avs[h][:, qlo:], lhsT=v_aug[:, kb, hh, :],
                                 rhs=et3[:, h, qlo:],
                                 start=(issued[0] == 0),
                                 stop=(issued[0] == nkb - 1))
            issued[0] += 1

        for n, kb in enumerate(kbs):
            r = kb - 4 * j  # >= 0 on diagonal blocks
            qlo = P * r if r >= 0 else 0
            ps_s = ps_sc.tile([P, 2 * SC], F32, tag="sc", name=f"pss{p}_{j}_{kb}")
            for h in range(2):
                hb = slice(DK * h, DK * (h + 1))
                nc.tensor.matmul(
                    ps_s[:, SC * h + qlo:SC * (h + 1)],
                    lhsT=kT_pair[p][hb, P * kb:P * (kb + 1)],
                    rhs=qT_pair[p][hb, SC * j + qlo:SC * (j + 1)],
                    start=True, stop=True)
            et = exp_pool.tile([P, 2 * SC], BF16, tag="exp", name=f"et{p}_{j}_{kb}")
            ps3 = ps_s.rearrange("p (h w) -> p h w", w=SC)
            et3 = et.rearrange("p (h w) -> p h w", w=SC)
            nc.scalar.activation(et3[:, :, qlo:], ps3[:, :, qlo:],
                                 mybir.ActivationFunctionType.Exp, scale=0.125)
            if r >= 0:
                # only the 128-wide triangle needs masking; beyond it every
                # key of this block is causally valid
                nc.vector.tensor_mul(
                    et3[:, :, qlo:qlo + P], et3[:, :, qlo:qlo + P],
                    mask128[:, 0:1, :].to_broadcast((P, 2, P)))
            pending.append((kb, qlo, et))
            while len(pending) > 3:
                issue_av(pending.pop(0))
            if early_cb is not None and n == 2:
                early_cb()
                early_cb = None
            f = slots.get(n)
            if f is not None:
                f()
        while pending:
            issue_av(pending.pop(0))
        if early_cb is not None:
            early_cb()

        def normalize():
            for h in range(2):
                sums = small.tile([1, SC], F32, tag="sums", name=f"sm{p}_{j}_{h}")
                nc.vector.reciprocal_approx_fast(sums[0:1, :], avs[h][0:1, :])
                rb = small.tile([P, SC], F32, tag="rb", name=f"rb{p}_{j}_{h}")
                nc.gpsimd.partition_broadcast(rb[:], sums[0:1, :])
                nc.vector.tensor_mul(att[h][DK:P, SC * j:SC * (j + 1)],
                                     avs[h][DK:P, :], rb[DK:P, :])
        return normalize

    # agt_all[:, i, :]: gathered attention outputs (out-proj lhsT); i = d-block
    # of the full 1024-dim attention output, i = 4*g + pair
    agt_all = const.tile([P, NI, S], BF16, name="agt_all")
    agt4 = agt_all.rearrange("p (g q) s -> p g q s", q=4)
    # out-proj partials from phase A (blocks 0,1,4,5,2,6), bf16, bo included
    part_lo = const.tile([P, NI, SC], BF16, tag="wq", name="part_lo")
    part_hi = const.tile([P, NI, SC], BF16, tag="wk", name="part_hi")

    def part_slice(qt):
        t = part_lo if qt < 8 else part_hi
        return t[:, qt % 8, :]

    def outproj_a(qt):
        """Phase A partial for q-tile qt (filler during pair 3): blocks
        0,1,4,5 (pairs 0/1) plus 2,6 (pair 2) -- all gathered well before
        their filler slot.  Phase B is then just 2 matmuls."""
        ps_o = ps_qk.tile([P, DG], F32, tag="psqk", name=f"psoa{qt}")
        for n, i in enumerate([0, 1, 4, 5, 2, 6]):
            nc.tensor.matmul(ps_o[:], lhsT=agt_all[:, i, P * qt:P * (qt + 1)],
                             rhs=wo[:, i, :], start=(n == 0), stop=(n == 5))
        nc.vector.tensor_add(part_slice(qt), ps_o[:], bo_t)

    def outproj_b(qt):
        """Phase B: blocks 3,7 + phase-A partial -> out (bf16 store)."""
        ps_o = ps_qk.tile([P, DG], F32, tag="psqk", name=f"psob{qt}")
        for n, i in enumerate([3, 7]):
            nc.tensor.matmul(ps_o[:], lhsT=agt_all[:, i, P * qt:P * (qt + 1)],
                             rhs=wo[:, i, :], start=(n == 0), stop=(n == 1))
        ot = out_pool.tile([P, DG], BF16, tag="ot", name=f"ot{qt}")
        nc.vector.tensor_add(ot[:], ps_o[:], part_slice(qt))
        eng = nc.sync if qt % 2 == 0 else nc.scalar
        eng.dma_start(out[P * qt:P * (qt + 1), :], ot[:])

    def send_chunk(p, j):
        """agin store + AllGather trigger for pair p, chunk j.  The agt
        reload is deferred (agt_load) so a waiting DMA never blocks the sync
        queue behind an in-flight collective."""
        csl = slice(SC * j, SC * (j + 1))
        nc.sync.dma_start(agin[p, j, 0:DK], att_tiles[p][0][DK:P, csl])
        nc.sync.dma_start(agin[p, j, DK:P], att_tiles[p][1][DK:P, csl])
        nc.gpsimd.collective_compute(
            "AllGather", mybir.AluOpType.bypass, replica_groups=groups,
            ins=[agin[p, j][:].opt()], outs=[agout[p, j][:].opt()])

    def agt_load(p, j):
        csl = slice(SC * j, SC * (j + 1))
        nc.sync.dma_start(agt4[:, :, p, csl],
                          agout[p, j].rearrange("g p s -> p g s"))

    # ---- prologue: QKV projections (qk ahead of v to match DMA arrival) ----
    qk_chunk(0, 0)
    qk_chunk(0, 1)
    v_chunk(0)
    qk_chunk(0, 2)
    v_chunk(1)
    qk_chunk(0, 3)
    v_chunk(2)
    v_chunk(3)

    # ---- attention, pair-pipelined, per-(pair, chunk) AllGathers ----
    att_tiles = []
    for p in range(NPAIR):
        att_tiles.append(
            [att_pool.tile([P, S], BF16, tag=f"att{h}", name=f"att{p}_{h}")
             for h in range(2)])

    # pair-3 filler schedule (ascending chunks; each chunk's AllGather fires
    # at block 2 of the NEXT chunk, so consumers sit >= one chunk behind):
    #   j0 (4 blocks):  phase A 0-1
    #   j1 (8 blocks):  phase A 2-8
    #   j2 (12 blocks): phase A 9-11, phase B 0-3 (chunk-0 AG done mid-j1)
    #   j3 (16 blocks): phase B 4-7 (chunk-1 AG done mid-j2), then 8-11
    #                   (chunk-2 AG fires at j3 block 2)
    #   epilogue: phase A 12-15 covers the chunk-3 AG latency, phase B 12-15
    p3_slots = {
        0: {n: (lambda qt=n - 2: outproj_a(qt)) for n in range(2, 4)},
        1: {n: (lambda qt=n + 1: outproj_a(qt)) for n in range(1, 7)},
        2: {**{n: (lambda qt=7 + n: outproj_a(qt)) for n in range(1, 5)},
            **{n: (lambda qt=n - 7: outproj_b(qt)) for n in range(7, 11)}},
        3: {**{n: (lambda qt=3 + n: outproj_b(qt)) for n in range(1, 5)},
            9: (lambda: agt_load(3, 2)),
            **{n: (lambda qt=n - 2: outproj_b(qt)) for n in range(10, 14)}},
    }

    norm_prev = None
    send_prev = None
    for p in range(NPAIR):
        for j in range(NSC):
            if p == 3 and j == 0:
                # all earlier pairs' gathers completed long ago: reload with
                # zero queue blocking
                for pp, jj in [(0, 0), (1, 0), (2, 0), (0, 1), (1, 1), (2, 1),
                               (0, 2), (1, 2), (0, 3), (1, 3)]:
                    agt_load(pp, jj)
            if p == 3 and j == 2:
                for pp, jj in [(2, 2), (2, 3), (3, 0)]:
                    agt_load(pp, jj)
            if p == 3 and j == 3:
                agt_load(3, 1)
            if p < 3:
                slots = {1: (lambda pp=p + 1, sc=2 * j: qk_chunk(pp, sc)),
                         3: (lambda pp=p + 1, sc=2 * j + 1: qk_chunk(pp, sc))} \
                    if j < 2 else {}
            else:
                slots = p3_slots[j]
            if norm_prev is not None:
                def early(nj=norm_prev, sp=send_prev):
                    nj()
                    send_chunk(*sp)
            else:
                early = None
            norm_prev = attention_chunk(p, j, att_tiles[p], slots, early)
            send_prev = (p, j)
    # epilogue: last chunk's norm + AG; phase A 12-15 hides the AG latency
    norm_prev()
    send_chunk(3, 3)
    agt_load(3, 3)
    for qt in range(12, 16):
        outproj_a(qt)
    for qt in range(12, 16):
        outproj_b(qt)

    for cm in reversed(ctxs):
        cm.__exit__(None, None, None)


def _prep_in_maps(x, Wq, bq, Wk, bk, Wv, bv, Wo, bo):
    bf16 = ml_dtypes.bfloat16
    in_maps = []
    k_idx = np.arange(P)[:, None]
    q_idx = np.arange(KB)[None, :]
    mask = (q_idx >= k_idx).astype(bf16)  # [128, 128]
    for c in range(8):
        b, g = divmod(c, 2)
        dsl = slice(g * DG, (g + 1) * DG)
        bc = np.empty((P, 8 + 2 * DG), dtype=np.float32)
        bc[:, 0:4] = bq[dsl].reshape(NPAIR, P).T
        bc[:, 4:8] = bk[dsl].reshape(NPAIR, P).T
        bc[:, 8:8 + DG] = np.broadcast_to(bv[dsl], (P, DG))
        bc[:, 8 + DG:] = np.broadcast_to(bo[dsl], (P, DG))
        in_maps.append({
            "xT": np.ascontiguousarray(x[b].T).astype(bf16),
            "wqT": np.ascontiguousarray(Wq[dsl].T).astype(bf16),
            "wkT": np.ascontiguousarray(Wk[dsl].T).astype(bf16),
            "wvT": np.ascontiguousarray(Wv[dsl].T).astype(bf16),
            "woT": np.ascontiguousarray(Wo[dsl].T).astype(bf16),
            "bconst": bc,
            "maskd": mask,
        })
    return in_maps


def kernel(x, Wq, bq, Wk, bk, Wv, bv, Wo, bo, _trace=False, _trace_kwargs=None):
    x, Wq, bq, Wk, bk = map(np.asarray, (x, Wq, bq, Wk, bk))
    Wv, bv, Wo, bo = map(np.asarray, (Wv, bv, Wo, bo))
    if "nc" not in _cache:
        _cache["nc"] = _build()
    nc = _cache["nc"]
    in_maps = _prep_in_maps(x, Wq, bq, Wk, bk, Wv, bv, Wo, bo)
    res = bass_utils.run_bass_kernel_spmd(
        nc, in_maps, core_ids=list(range(8)), trace=_trace,
        **(_trace_kwargs or {}))
    _cache["last_result"] = res
    out = np.empty((B, S, D), dtype=np.float32)
    for c in range(8):
        b, g = divmod(c, 2)
        out[b, :, g * DG:(g + 1) * DG] = np.asarray(
            res.results[c]["out"], dtype=np.float32)
    return out


# revision 22
# speedup vs baseline: 1.1305x; 1.0332x over previous
"""Causal multi-head attention on 8 TRN2 NeuronCores.

Sharding: core c = (batch b=c//2, head-group g=c%2). Each core computes QKV
projections + causal attention for its 8 heads over the full sequence of its
batch; 2-rank AllGathers (pair shares a batch) exchange attention outputs;
each core then runs the output projection for its half of the output columns.

Perf structure (v2):
  - DMAs consolidated into multi-dim-AP transfers and split across the two
    HW DGE queues (sync + scalar) so the PE never waits on a serialized
    descriptor chain.
  - v_aug ones/zeros pattern built by gpsimd memsets (no DMA).
  - Causal mask multiply shrunk to the 128-wide diagonal triangle (the mask
    is identical for every diagonal block).
  - Softmax normalize runs early (emitted at block 2 of the following chunk)
    straight from PSUM -- no staging copy; reciprocal on DVE, partition
    broadcast on gpsimd.
  - Last pair processes q-chunks in descending order with one AllGather per
    chunk; output projection phase A covers blocks [0,1,4,5,2,6] so phase B
    is just [3,7]+store, emitted as fillers as each AllGather lands.
  - Output stored as bf16, stores alternate between the two DMA queues.
"""

import numpy as np
import ml_dtypes

import concourse.bass as bass
import concourse.mybir as mybir
import concourse.tile as tile
from concourse import bacc
from concourse import bass_utils

BF16 = mybir.dt.bfloat16
F32 = mybir.dt.float32

B, S, D = 4, 2048, 1024
H, DK = 16, 64
HPG = 8          # heads per group (per core)
DG = HPG * DK    # 512, d-range per core
NPAIR = 4        # head pairs per core
SC = 512         # sequence chunk (matmul free dim)
NSC = S // SC    # 4
KB = 128         # key block
NKB = S // KB    # 16
P = 128
NI = D // P      # 8

_cache = {}
DEBUG = False


def _build():
    nc = bacc.Bacc("TRN2", target_bir_lowering=False, debug=False, num_devices=8)

    xT = nc.dram_tensor("xT", [D, S], BF16, kind="ExternalInput")
    wqT = nc.dram_tensor("wqT", [D, DG], BF16, kind="ExternalInput")
    wkT = nc.dram_tensor("wkT", [D, DG], BF16, kind="ExternalInput")
    wvT = nc.dram_tensor("wvT", [D, DG], BF16, kind="ExternalInput")
    woT = nc.dram_tensor("woT", [D, DG], BF16, kind="ExternalInput")
    # bconst: bq [0:4], bk [4:8], bv broadcast [8:520], bo broadcast [520:1032]
    bconst = nc.dram_tensor("bconst", [P, 8 + 2 * DG], F32, kind="ExternalInput")
    maskd = nc.dram_tensor("maskd", [P, KB], BF16, kind="ExternalInput")
    out = nc.dram_tensor("out", [S, DG], BF16, kind="ExternalOutput")

    with tile.TileContext(nc) as tc:
        _emit(nc, tc, xT, wqT, wkT, wvT, woT, bconst, maskd, out)
    nc.compile()
    return nc


def _emit(nc, tc, xT, wqT, wkT, wvT, woT, bconst_d, maskd, out):
    ctxs = []

    def pool(name, bufs, space="SBUF"):
        cm = tc.tile_pool(name=name, bufs=bufs, space=space)
        p = cm.__enter__()
        ctxs.append(cm)
        return p

    const = pool("const", 1)
    dram = pool("dram", 1, space="DRAM")
    qk_pool = pool("qk", 2)
    att_pool = pool("att", 2)
    exp_pool = pool("exp", 5)
    small = pool("small", 3)
    out_pool = pool("outp", 2)
    ps_qk = pool("ps_qk", 2, space="PSUM")
    ps_sc = pool("ps_sc", 2, space="PSUM")
    ps_av = pool("ps_av", 2, space="PSUM")

    # ---- constants / weights ----
    xt = const.tile([P, NI, S], BF16, name="xt")
    wq = const.tile([P, NI, DG], BF16, name="wq")
    wk = const.tile([P, NI, DG], BF16, name="wk")
    wv = const.tile([P, NI, DG], BF16, name="wv")
    wo = const.tile([P, NI, DG], BF16, name="wo")
    bconst = const.tile([P, 8 + 2 * DG], F32, name="bconst")
    mask128 = const.tile([P, 1, KB], BF16, name="mask128")
    # v_aug[:, t, h, :]: col 0 = ones (softmax denominator row), cols 1:64 =
    # zeros (PSUM partition alignment pad), cols 64:128 = v
    v_aug = const.tile([P, NKB, HPG, P], BF16, name="v_aug")

    # weights on the scalar HW DGE queue, x on the sync queue (first-needed
    # tiles in small transfers for latency, the rest consolidated); small
    # constants via gpsimd SW DGE so they don't queue behind the weights
    wqr = wqT.rearrange("(i p) d -> p i d", p=P)
    wkr = wkT.rearrange("(i p) d -> p i d", p=P)
    xTr = xT.rearrange("(i p) s -> p i s", p=P)
    for i in range(0, NI, 2):
        nc.scalar.dma_start(wq[:, i:i + 2, :], wqr[:, i:i + 2, :])
    for i in range(0, NI, 2):
        nc.sync.dma_start(xt[:, i:i + 2, 0:SC], xTr[:, i:i + 2, 0:SC])
    for i in range(0, NI, 4):
        nc.scalar.dma_start(wk[:, i:i + 4, :], wkr[:, i:i + 4, :])
    nc.sync.dma_start(xt[:, :, SC:2 * SC], xTr[:, :, SC:2 * SC])
    nc.sync.dma_start(wv[:], wvT.rearrange("(i p) d -> p i d", p=P))
    for sc in range(2, NSC):
        nc.sync.dma_start(xt[:, :, SC * sc:SC * (sc + 1)],
                          xTr[:, :, SC * sc:SC * (sc + 1)])
    nc.scalar.dma_start(wo[:], woT.rearrange("(i p) d -> p i d", p=P))
    nc.gpsimd.dma_start(bconst[:], bconst_d[:])
    nc.gpsimd.dma_start(mask128[:], maskd[:])

    bq_t = bconst[:, 0:4]
    bk_t = bconst[:, 4:8]
    bv_t = bconst[:, 8:8 + DG]
    bo_t = bconst[:, 8 + DG:8 + 2 * DG]

    # DRAM bounce buffers for the per-(pair, chunk) AllGathers.  One small
    # collective per chunk keeps every AllGather far ahead of its consumer;
    # the first one (pair 0 chunk 0) absorbs the cross-core launch skew
    # ~100us before anything reads gathered data.
    agin = dram.tile([NPAIR, NSC, P, SC], BF16, name="agin")
    agout = dram.tile([NPAIR, NSC, 2, P, SC], BF16, name="agout")
    agin_p = dram.tile([3, P, S], BF16, name="agin_p")
    agout_p = dram.tile([3, 2, P, S], BF16, name="agout_p")
    dummy_in = dram.tile([P, 16], BF16, name="dummy_in")
    dummy_out = dram.tile([2, P, 16], BF16, name="dummy_out")

    groups = [[0, 1], [2, 3], [4, 5], [6, 7]]

    # Tiny dummy AllGather issued at kernel start: the collective stream runs
    # in order and each trigger waits for the previous collective, so the
    # FIRST collective absorbs the cross-core launch skew (tens of us).  Fire
    # it here, where nothing depends on the gpsimd queue for a long time, so
    # every real AllGather later completes in a few us.
    nc.gpsimd.collective_compute(
        "AllGather", mybir.AluOpType.bypass, replica_groups=groups,
        ins=[dummy_in[:].opt()], outs=[dummy_out[:].opt()])

    # v_aug constant pattern via gpsimd (keeps both DMA queues clear)
    for t in range(NKB):
        nc.gpsimd.memset(v_aug[:, t, :, 1:DK], 0.0)
        nc.gpsimd.memset(v_aug[:, t, :, 0:1], 1.0)

    qT_pair = [qk_pool.tile([P, S], BF16, tag="qT", name=f"qTp{pp}")
               for pp in range(NPAIR)]
    kT_pair = [qk_pool.tile([P, S], BF16, tag="kT", name=f"kTp{pp}")
               for pp in range(NPAIR)]

    def qk_chunk(p, sc):
        """q/k projections for pair p, seq chunk sc."""
        ssl = slice(SC * sc, SC * (sc + 1))
        ps_q = ps_qk.tile([P, SC], F32, tag="psqk", name=f"psq{p}_{sc}")
        for i in range(NI):
            nc.tensor.matmul(ps_q[:], lhsT=wq[:, i, P * p:P * (p + 1)],
                             rhs=xt[:, i, ssl], start=(i == 0), stop=(i == 7))
        nc.vector.tensor_add(qT_pair[p][:, ssl], ps_q[:],
                             bq_t[:, p:p + 1].to_broadcast((P, SC)))
        ps_k = ps_qk.tile([P, SC], F32, tag="psqk", name=f"psk{p}_{sc}")
        for i in range(NI):
            nc.tensor.matmul(ps_k[:], lhsT=wk[:, i, P * p:P * (p + 1)],
                             rhs=xt[:, i, ssl], start=(i == 0), stop=(i == 7))
        nc.vector.tensor_add(kT_pair[p][:, ssl], ps_k[:],
                             bk_t[:, p:p + 1].to_broadcast((P, SC)))

    def v_chunk(sc):
        """v projection for seq chunk sc (all 8 heads), into v_aug."""
        for st in range(4):
            t = 4 * sc + st
            ps_v = ps_qk.tile([P, DG], F32, tag="psqk", name=f"psv{sc}_{st}")
            for i in range(NI):
                nc.tensor.matmul(ps_v[:], lhsT=xt[:, i, P * t:P * (t + 1)],
                                 rhs=wv[:, i, :], start=(i == 0), stop=(i == 7))
            nc.vector.tensor_add(v_aug[:, t, :, DK:P],
                                 ps_v[:].rearrange("p (h c) -> p h c", c=DK),
                                 bv_t.rearrange("p (h c) -> p h c", c=DK))

    def attention_chunk(p, j, att, slots, early_cb):
        """Causal attention for head pair p, q chunk j. Both heads row-packed
        into one wide psum; one wide exp; diag blocks first. slots maps block
        index -> filler callable; early_cb (prev chunk's normalize+AG) is
        emitted at block 2."""
        avs = [ps_av.tile([P, SC], F32, tag="av", name=f"av{p}_{j}_{h}")
               for h in range(2)]
        nkb = 4 * (j + 1)
        kbs = list(range(4 * j, nkb)) + list(range(0, 4 * j))  # diag first
        pending = []
        issued = [0]

        def issue_av(item):
            kb, qlo, et = item
            et3 = et.rearrange("p (h w) -> p h w", w=SC)
            for h in range(2):
                hh = 2 * p + h
                nc.tensor.matmul(avs[h][:, qlo:], lhsT=v_aug[:, kb, hh, :],
                                 rhs=et3[:, h, qlo:],
                                 start=(issued[0] == 0),
                                 stop=(issued[0] == nkb - 1))
            issued[0] += 1

        for n, kb in enumerate(kbs):
            r = kb - 4 * j  # >= 0 on diagonal blocks
            qlo = P * r if r >= 0 else 0
            ps_s = ps_sc.tile([P, 2 * SC], F32, tag="sc", name=f"pss{p}_{j}_{kb}")
            for h in range(2):
                hb = slice(DK * h, DK * (h + 1))
                nc.tensor.matmul(
                    ps_s[:, SC * h + qlo:SC * (h + 1)],
                    lhsT=kT_pair[p][hb, P * kb:P * (kb + 1)],
                    rhs=qT_pair[p][hb, SC * j + qlo:SC * (j + 1)],
                    start=True, stop=True)
            et = exp_pool.tile([P, 2 * SC], BF16, tag="exp", name=f"et{p}_{j}_{kb}")
            ps3 = ps_s.rearrange("p (h w) -> p h w", w=SC)
            et3 = et.rearrange("p (h w) -> p h w", w=SC)
            nc.scalar.activation(et3[:, :, qlo:], ps3[:, :, qlo:],
                                 mybir.ActivationFunctionType.Exp, scale=0.125)
            if r >= 0:
                # only the 128-wide triangle needs masking; beyond it every
                # key of this block is causally valid
                nc.vector.tensor_mul(
                    et3[:, :, qlo:qlo + P], et3[:, :, qlo:qlo + P],
                    mask128[:, 0:1, :].to_broadcast((P, 2, P)))
            pending.append((kb, qlo, et))
            while len(pending) > 3:
                issue_av(pending.pop(0))
            if early_cb is not None and n == 2:
                early_cb()
                early_cb = None
            f = slots.get(n)
            if f is not None:
                f()
        while pending:
            issue_av(pending.pop(0))
        if early_cb is not None:
            early_cb()

        def normalize():
            for h in range(2):
                sums = small.tile([1, SC], F32, tag="sums", name=f"sm{p}_{j}_{h}")
                nc.vector.reciprocal_approx_fast(sums[0:1, :], avs[h][0:1, :])
                rb = small.tile([P, SC], F32, tag="rb", name=f"rb{p}_{j}_{h}")
                nc.gpsimd.partition_broadcast(rb[:], sums[0:1, :])
                nc.vector.tensor_mul(att[h][DK:P, SC * j:SC * (j + 1)],
                                     avs[h][DK:P, :], rb[DK:P, :])
        return normalize

    # agt_all[:, i, :]: gathered attention outputs (out-proj lhsT); i = d-block
    # of the full 1024-dim attention output, i = 4*g + pair
    agt_all = const.tile([P, NI, S], BF16, name="agt_all")
    agt4 = agt_all.rearrange("p (g q) s -> p g q s", q=4)
    # out-proj partials from phase A (blocks 0,1,4,5,2,6), bf16, bo included
    part_lo = const.tile([P, NI, SC], BF16, tag="wq", name="part_lo")
    part_hi = const.tile([P, NI, SC], BF16, tag="wk", name="part_hi")

    def part_slice(qt):
        t = part_lo if qt < 8 else part_hi
        return t[:, qt % 8, :]

    def outproj_a(qt):
        """Phase A partial for q-tile qt: blocks 0,1,4,5 (pairs 0/1, gathered
        two pairs before pair 3 runs)."""
        ps_o = ps_qk.tile([P, DG], F32, tag="psqk", name=f"psoa{qt}")
        for n, i in enumerate([0, 1, 4, 5]):
            nc.tensor.matmul(ps_o[:], lhsT=agt_all[:, i, P * qt:P * (qt + 1)],
                             rhs=wo[:, i, :], start=(n == 0), stop=(n == 3))
        nc.vector.tensor_add(part_slice(qt), ps_o[:], bo_t)

    def outproj_b(qt):
        """Phase B: blocks 2,6 (pair 2), 3,7 (pair 3) + partial -> out."""
        ps_o = ps_qk.tile([P, DG], F32, tag="psqk", name=f"psob{qt}")
        for n, i in enumerate([2, 6, 3, 7]):
            nc.tensor.matmul(ps_o[:], lhsT=agt_all[:, i, P * qt:P * (qt + 1)],
                             rhs=wo[:, i, :], start=(n == 0), stop=(n == 3))
        ot = out_pool.tile([P, DG], BF16, tag="ot", name=f"ot{qt}")
        nc.vector.tensor_add(ot[:], ps_o[:], part_slice(qt))
        eng = nc.sync if qt % 2 == 0 else nc.scalar
        eng.dma_start(out[P * qt:P * (qt + 1), :], ot[:])

    def send_pair(p):
        """Full-sequence agin store + AllGather for pairs 0-2.  Few, widely
        spaced collectives keep the in-order CC stream from backing up (a
        blocked trigger stalls the gpsimd queue and the softmax broadcasts
        behind it)."""
        nc.sync.dma_start(agin_p[p, 0:DK], att_tiles[p][0][DK:P, :])
        nc.sync.dma_start(agin_p[p, DK:P], att_tiles[p][1][DK:P, :])
        nc.gpsimd.collective_compute(
            "AllGather", mybir.AluOpType.bypass, replica_groups=groups,
            ins=[agin_p[p][:].opt()], outs=[agout_p[p][:].opt()])

    def agt_load_pair(p):
        nc.sync.dma_start(agt4[:, :, p, :],
                          agout_p[p].rearrange("g p s -> p g s"))

    def send_chunk(p, j):
        """Per-chunk agin store + AllGather trigger (pair 3 only)."""
        csl = slice(SC * j, SC * (j + 1))
        nc.sync.dma_start(agin[p, j, 0:DK], att_tiles[p][0][DK:P, csl])
        nc.sync.dma_start(agin[p, j, DK:P], att_tiles[p][1][DK:P, csl])
        nc.gpsimd.collective_compute(
            "AllGather", mybir.AluOpType.bypass, replica_groups=groups,
            ins=[agin[p, j][:].opt()], outs=[agout[p, j][:].opt()])

    def agt_load(p, j):
        csl = slice(SC * j, SC * (j + 1))
        nc.sync.dma_start(agt4[:, :, p, csl],
                          agout[p, j].rearrange("g p s -> p g s"))

    # ---- prologue: QKV projections (qk ahead of v to match DMA arrival) ----
    qk_chunk(0, 0)
    qk_chunk(0, 1)
    v_chunk(0)
    qk_chunk(0, 2)
    v_chunk(1)
    qk_chunk(0, 3)
    v_chunk(2)
    v_chunk(3)

    # ---- attention, pair-pipelined, per-(pair, chunk) AllGathers ----
    att_tiles = []
    for p in range(NPAIR):
        att_tiles.append(
            [att_pool.tile([P, S], BF16, tag=f"att{h}", name=f"att{p}_{h}")
             for h in range(2)])

    # pair-3 filler schedule (ascending chunks; each chunk's AllGather fires
    # at block 2 of the NEXT chunk, so consumers sit >= one chunk behind):
    #   j0 (4 blocks):  phase A 0-1
    #   j1 (8 blocks):  phase A 2-8
    #   j2 (12 blocks): phase A 9-11, phase B 0-3 (chunk-0 AG done mid-j1)
    #   j3 (16 blocks): phase B 4-7 (chunk-1 AG done mid-j2), then 8-11
    #                   (chunk-2 AG fires at j3 block 2)
    #   epilogue: phase A 12-15 covers the chunk-3 AG latency, phase B 12-15
    p3_slots = {
        0: {n: (lambda qt=n - 2: outproj_a(qt)) for n in range(2, 4)},
        1: {n: (lambda qt=n + 1: outproj_a(qt)) for n in range(1, 7)},
        2: {**{n: (lambda qt=7 + n: outproj_a(qt)) for n in range(1, 5)},
            **{n: (lambda qt=n - 7: outproj_b(qt)) for n in range(7, 11)}},
        3: {**{n: (lambda qt=2 + n: outproj_b(qt)) for n in range(2, 6)},
            9: (lambda: agt_load(3, 2)),
            **{n: (lambda qt=n - 2: outproj_b(qt)) for n in range(10, 14)}},
    }

    norm_prev = None
    for p in range(NPAIR):
        for j in range(NSC):
            if p == 3 and j == 0:
                agt_load_pair(0)
                agt_load_pair(1)
            if p == 3 and j == 2:
                agt_load_pair(2)
                agt_load(3, 0)
            if p == 3 and j == 3:
                agt_load(3, 1)
            if p < 3:
                slots = {1: (lambda pp=p + 1, sc=2 * j: qk_chunk(pp, sc)),
                         3: (lambda pp=p + 1, sc=2 * j + 1: qk_chunk(pp, sc))} \
                    if j < 2 else {}
            else:
                slots = p3_slots[j]
            if norm_prev is not None:
                # previous chunk's normalize; at a pair boundary also its
                # pair's AllGather (pair 3: per-chunk AllGathers)
                pprev, jprev = (p, j - 1) if j > 0 else (p - 1, 3)
                def early(nj=norm_prev, pp=pprev, jj=jprev):
                    nj()
                    if pp == 3:
                        send_chunk(pp, jj)
                    elif jj == 3:
                        send_pair(pp)
            else:
                early = None
            norm_prev = attention_chunk(p, j, att_tiles[p], slots, early)
    # epilogue: last chunk's norm + AG; phase A 12-15 hides the AG latency
    norm_prev()
    send_chunk(3, 3)
    agt_load(3, 3)
    for qt in range(12, 16):
        outproj_a(qt)
    for qt in range(12, 16):
        outproj_b(qt)

    for cm in reversed(ctxs):
        cm.__exit__(None, None, None)


def _prep_in_maps(x, Wq, bq, Wk, bk, Wv, bv, Wo, bo):
    bf16 = ml_dtypes.bfloat16
    in_maps = []
    k_idx = np.arange(P)[:, None]
    q_idx = np.arange(KB)[None, :]
    mask = (q_idx >= k_idx).astype(bf16)  # [128, 128]
    for c in range(8):
        b, g = divmod(c, 2)
        dsl = slice(g * DG, (g + 1) * DG)
        bc = np.empty((P, 8 + 2 * DG), dtype=np.float32)
        bc[:, 0:4] = bq[dsl].reshape(NPAIR, P).T
        bc[:, 4:8] = bk[dsl].reshape(NPAIR, P).T
        bc[:, 8:8 + DG] = np.broadcast_to(bv[dsl], (P, DG))
        bc[:, 8 + DG:] = np.broadcast_to(bo[dsl], (P, DG))
        in_maps.append({
            "xT": np.ascontiguousarray(x[b].T).astype(bf16),
            "wqT": np.ascontiguousarray(Wq[dsl].T).astype(bf16),
            "wkT": np.ascontiguousarray(Wk[dsl].T).astype(bf16),
            "wvT": np.ascontiguousarray(Wv[dsl].T).astype(bf16),
            "woT": np.ascontiguousarray(Wo[dsl].T).astype(bf16),
            "bconst": bc,
            "maskd": mask,
        })
    return in_maps


def kernel(x, Wq, bq, Wk, bk, Wv, bv, Wo, bo, _trace=False, _trace_kwargs=None):
    x, Wq, bq, Wk, bk = map(np.asarray, (x, Wq, bq, Wk, bk))
    Wv, bv, Wo, bo = map(np.asarray, (Wv, bv, Wo, bo))
    if "nc" not in _cache:
        _cache["nc"] = _build()
    nc = _cache["nc"]
    in_maps = _prep_in_maps(x, Wq, bq, Wk, bk, Wv, bv, Wo, bo)
    res = bass_utils.run_bass_kernel_spmd(
        nc, in_maps, core_ids=list(range(8)), trace=_trace,
        **(_trace_kwargs or {}))
    _cache["last_result"] = res
    out = np.empty((B, S, D), dtype=np.float32)
    for c in range(8):
        b, g = divmod(c, 2)
        out[b, :, g * DG:(g + 1) * DG] = np.asarray(
            res.results[c]["out"], dtype=np.float32)
    return out
